# revision 44
# baseline (speedup 1.0000x reference)
"""AnomalyDINO kNN retrieval kernel for one TRN2 chip (8 NeuronCores).

Strategy (sharded-kNN):
  - memory bank rows (M=65536) sharded 8-ways: each core owns 8192 rows
  - every core computes max-cosine-sim of ALL 8192 patches (B*N) against its
    bank shard: operands l2-normalized on device (f32 norms), cast to fp8
    e4m3, contraction padded 384->512 and run as 2 DoubleRow matmuls per
    512-wide output (f32 PSUM accumulate); VectorE reduce_max per
    [128,1024] PSUM group collects per-patch maxima
  - two ReduceScatter(max) collectives (16KB each) combine the 8 partial
    maxima and hand core c exactly the 1024 patches of image c (patch
    p=b*1024+n); the patch loop is ordered so grid-tiles j<4 of every image
    finish first, letting the first collective run under the second half of
    the matmul work (the halves are routed so RS chunk c = image c's half)
  - each core finishes its own image on-device: dist = clip(1-sim,0,2),
    pred_score = mean(top-10 of 1024) via Max8 + MatchReplace + Max8,
    anomaly map = L @ A(32x32) @ L^T (f32 TensorE matmuls) where
    L = (gaussian-blur-448 @ bilinear-resize-448x32) is a host-precomputed
    constant (exact linear operator for jax.image.resize + kornia-style
    reflect-pad blur, verified to 1.4e-6 against the jax reference)

fp8 e4m3 end-to-end rel err vs the f32 reference: 6.7e-3 (gate 2e-2).
"""

import functools
import sys

import numpy as np

B, N, D = 8, 1024, 384
M = 65536
NCORES = 8
MSH = M // NCORES        # bank rows per core
P = 128                  # SBUF partitions
NPT = (B * N) // P       # 64 patch tiles
FREE = 512               # matmul moving free dim (one PSUM bank of f32)
GFREE = 1024             # PSUM group width (2 banks, reduced by one DVE op)
NBT = MSH // GFREE       # 8 bank-tile groups per core
KC = D // P              # 3 contraction chunks
GRID = 32
IMG = 448
SIGMA = 4.0
NUM_TOP = 10


def _build_lt() -> np.ndarray:
    """LT = (Blur448 @ Resize448x32)^T as float32 [32, 448].

    Resize: jax.image.resize bilinear (half-pixel centers, clamped edges).
    Blur: sigma=4, radius=16 separable gaussian with reflect padding.
    """
    scale = GRID / IMG
    R = np.zeros((IMG, GRID), dtype=np.float64)
    for i in range(IMG):
        u = (i + 0.5) * scale - 0.5
        lo = int(np.floor(u))
        frac = u - lo
        for j, w in ((lo, 1.0 - frac), (lo + 1, frac)):
            R[i, min(max(j, 0), GRID - 1)] += w
    radius = int(4.0 * SIGMA + 0.5)
    t = np.arange(-radius, radius + 1, dtype=np.float64)
    k = np.exp(-0.5 * (t / SIGMA) ** 2)
    k /= k.sum()
    Bm = np.zeros((IMG, IMG), dtype=np.float64)
    for i in range(IMG):
        for tt in range(2 * radius + 1):
            j = i - radius + tt
            while j < 0 or j >= IMG:
                j = -j if j < 0 else 2 * (IMG - 1) - j
            Bm[i, j] += k[tt]
    return np.ascontiguousarray((Bm @ R).T.astype(np.float32))


@functools.lru_cache(maxsize=1)
def _get_nc():
    import os

    gfree = int(os.environ.get("KBENCH_GF", GFREE))
    nbt_all = MSH // gfree
    nbt = int(os.environ.get("KBENCH_NBT", nbt_all))
    nprep = int(os.environ.get("KBENCH_NPREP", 64))
    use_fp8 = os.environ.get("KBENCH_FP8", "1") == "1"
    use_ttr = os.environ.get("KBENCH_TTR", "0") == "1"
    use_ord = os.environ.get("KBENCH_ORD", "1") == "1"
    use_pad = os.environ.get("KBENCH_PAD", "1") == "1"
    kc = 4 if (use_fp8 and use_pad) else KC
    if "/opt/trn_rl_repo" not in sys.path:
        sys.path.insert(0, "/opt/trn_rl_repo")
    from concourse import bacc, masks, mybir, tile

    if os.environ.get("KBENCH_LDWOPT", "0") == "1":
        from concourse import bass_utils as _bu

        if not getattr(_bu, "_ldwopt_patched", False):
            _orig_run_command = _bu.run_command

            def _patched_run_command(argv, **kw):
                argv = [
                    "--enable-ldw-opt=true" if a == "--enable-ldw-opt=false" else a
                    for a in argv
                ]
                return _orig_run_command(argv, **kw)

            _bu.run_command = _patched_run_command
            _bu._ldwopt_patched = True

    dt = mybir.dt
    AX = mybir.AxisListType
    AF = mybir.ActivationFunctionType
    ALU = mybir.AluOpType

    nc = bacc.Bacc(
        "TRN2",
        target_bir_lowering=False,
        debug=False,
        enable_asserts=False,
        num_devices=NCORES,
    )

    f_ext = nc.dram_tensor("features", [B * N, D], dt.float32, kind="ExternalInput")
    mb_ext = nc.dram_tensor("mb", [MSH, D], dt.float32, kind="ExternalInput")
    lt_ext = nc.dram_tensor("lt", [GRID, IMG], dt.float32, kind="ExternalInput")
    amap_ext = nc.dram_tensor("amap", [IMG, IMG], dt.float32, kind="ExternalOutput")
    score_ext = nc.dram_tensor("score", [1, 1], dt.float32, kind="ExternalOutput")

    with tile.TileContext(nc) as tc:
        with (
            tc.tile_pool(name="persist", bufs=1) as persist,
            tc.tile_pool(name="io", bufs=4) as io,
            tc.tile_pool(name="stats", bufs=6) as stats,
            tc.tile_pool(
                name="psum_mm", bufs=6 // (gfree // FREE), space="PSUM"
            ) as psum_mm,
            tc.tile_pool(name="psum_tp", bufs=2, space="PSUM") as psum_tp,
            tc.tile_pool(name="dram", bufs=1, space="DRAM") as dram,
        ):
            if os.environ.get("KBENCH_BAR", "0") == "1":
                # align cores at kernel entry so the later collective's
                # rendezvous wait overlaps the DMA/prep phase
                nc.gpsimd.bir_kernel_barrier_wait([list(range(NCORES))])
            ident_b = persist.tile([P, P], dt.bfloat16, name="ident_b")
            masks.make_identity(nc, ident_b[:])
            ident_f = persist.tile([P, P], dt.float32, name="ident_f")
            masks.make_identity(nc, ident_f[:])
            lt_sb = persist.tile([GRID, IMG], dt.float32, name="lt_sb")
            nc.sync.dma_start(lt_sb[:], lt_ext[:])
            if os.environ.get("KBENCH_AG", "0") == "1":
                sel_ext = nc.dram_tensor(
                    "sel", [NPT, NCORES], dt.float32, kind="ExternalInput"
                )
                sel_sb = persist.tile([NPT, NCORES], dt.float32, name="sel_sb")
                nc.sync.dma_start(sel_sb[:], sel_ext[:])

            op_dt = dt.float8e4 if use_fp8 else dt.bfloat16
            fT3 = persist.tile([P, kc, B * N], op_dt, name="fT3")
            mbT3 = persist.tile([P, kc, MSH], op_dt, name="mbT3")
            if kc == 4:
                nc.gpsimd.memset(fT3[:, 3, :], 0.0)
                nc.gpsimd.memset(mbT3[:, 3, :], 0.0)
            maxall = persist.tile([P, NPT, nbt_all], dt.float32, name="maxall")
            gmax = persist.tile([P, NPT], dt.float32, name="gmax")

            prep_old = os.environ.get("KBENCH_PREPOLD") == "1"

            def prep(src, t, dstT):
                # load [128, 384] f32, l2-normalize rows, cast, transpose the
                # three 128x128 blocks onto the contraction axis
                ld = io.tile([P, D], dt.float32, name="ld", tag="ld")
                nc.sync.dma_start(ld[:], src[t * P : (t + 1) * P, :])
                sq = io.tile([P, D], dt.float32, name="sq", tag="sq")
                ss = stats.tile([P, 1], dt.float32, name="ss", tag="ss")
                nc.scalar.activation(sq[:], ld[:], AF.Square, accum_out=ss[:])
                nrm = stats.tile([P, 1], dt.float32, name="nrm", tag="nrm")
                nc.scalar.sqrt(nrm[:], ss[:])
                rin = stats.tile([P, 1], dt.float32, name="rin", tag="rin")
                nc.vector.reciprocal(rin[:], nrm[:])
                nb = io.tile([P, D], dt.bfloat16, name="nb", tag="nb")
                nc.scalar.activation(nb[:], ld[:], AF.Copy, scale=rin[:])
                for k in range(KC):
                    tp = psum_tp.tile([P, P], dt.bfloat16, name="tp", tag="tp")
                    nc.tensor.transpose(tp[:], nb[:, k * P : (k + 1) * P], ident_b[:])
                    nc.scalar.copy(dstT[:, k, t * P : (t + 1) * P], tp[:])

            PB = 4  # row-tiles per batched prep step

            def prep4(src_r, t4, dstT, eng):
                # batched prep: one DMA + fused normalize for PB row-tiles
                ld4 = io.tile([P, PB, D], dt.float32, name="ld4", tag="ld4")
                eng.dma_start(ld4[:], src_r[:, t4 * PB : (t4 + 1) * PB, :])
                sq4 = io.tile([P, PB, D], dt.float32, name="sq4", tag="sq4")
                nc.scalar.activation(sq4[:], ld4[:], AF.Square)
                ss4 = stats.tile([P, PB], dt.float32, name="ss4", tag="ss4")
                nc.vector.reduce_sum(ss4[:], sq4[:], AX.X)
                nrm4 = stats.tile([P, PB], dt.float32, name="nrm4", tag="nrm4")
                nc.scalar.sqrt(nrm4[:], ss4[:])
                rin4 = stats.tile([P, PB], dt.float32, name="rin4", tag="rin4")
                nc.vector.reciprocal(rin4[:], nrm4[:])
                nb4 = io.tile([P, PB, D], dt.bfloat16, name="nb4", tag="nb4")
                for j in range(PB):
                    nc.scalar.activation(
                        nb4[:, j, :], ld4[:, j, :], AF.Copy, scale=rin4[:, j : j + 1]
                    )
                for j in range(PB):
                    tp4 = psum_tp.tile([P, KC, P], dt.bfloat16, name="tp4", tag="tp")
                    for k in range(KC):
                        nc.tensor.transpose(
                            tp4[:, k, :], nb4[:, j, k * P : (k + 1) * P], ident_b[:]
                        )
                    t = t4 * PB + j
                    nc.scalar.copy(dstT[:, 0:KC, t * P : (t + 1) * P], tp4[:])

            if prep_old:
                for t in range(min(4, nprep)):
                    prep(mb_ext, t, mbT3)
                for t in range(nprep):
                    prep(f_ext, t, fT3)
                for t in range(4, nprep):
                    prep(mb_ext, t, mbT3)
            else:
                # rearranged [p, tile, d] views for batched loads
                f_r = f_ext[:].rearrange("(j p) d -> p j d", p=P)
                mb_r = mb_ext[:].rearrange("(j p) d -> p j d", p=P)
                nf4 = nprep // PB
                if os.environ.get("KBENCH_SPLIT", "1") == "2":
                    # bt-outer within half: first sweep needs mb tiles 0-7 and
                    # the even f groups
                    prep4(mb_r, 0, mbT3, nc.sync)
                    prep4(mb_r, 1, mbT3, nc.scalar)
                    for t4 in range(0, nf4, 2):
                        prep4(f_r, t4, fT3, nc.scalar if t4 % 2 else nc.sync)
                    for t4 in range(2, nf4):
                        prep4(mb_r, t4, mbT3, nc.scalar if t4 % 2 else nc.sync)
                    for t4 in range(1, nf4, 2):
                        prep4(f_r, t4, fT3, nc.scalar if t4 % 2 else nc.sync)
                elif os.environ.get("KBENCH_SPLIT", "1") != "0":
                    # split order consumes f tiles 0,8,16,.. first, and every
                    # sweep needs the whole bank: f(0), all mb, f evens, odds
                    prep4(f_r, 0, fT3, nc.sync)
                    for t4 in range(nf4):
                        prep4(mb_r, t4, mbT3, nc.scalar if t4 % 2 else nc.sync)
                    for t4 in list(range(2, nf4, 2)) + list(range(1, nf4, 2)):
                        prep4(f_r, t4, fT3, nc.scalar if t4 % 2 else nc.sync)
                else:
                    for t4 in range(min(1, nf4)):
                        prep4(mb_r, t4, mbT3, nc.sync)
                    for t4 in range(nf4):
                        prep4(f_r, t4, fT3, nc.scalar if t4 % 2 else nc.sync)
                    for t4 in range(1, nf4):
                        prep4(mb_r, t4, mbT3, nc.scalar if t4 % 2 else nc.sync)

            # main phase: (64 patch tiles) x (8 bank groups) x (2 halves);
            # fp8: k-chunks 0,1 as one DoubleRow matmul + chunk 2 plain (or
            # padded to 4 chunks = 2 DoubleRow matmuls)
            DR = mybir.MatmulPerfMode.DoubleRow

            def half_steps():
                # (k_slice, perf_mode, is_first, is_last) per accumulation step
                if not use_fp8:
                    return [
                        ((k, k + 1), None, k == 0, k == KC - 1) for k in range(KC)
                    ]
                if kc == 4:
                    return [((0, 2), DR, True, False), ((2, 4), DR, False, True)]
                return [((0, 2), DR, True, False), ((2, 3), None, False, True)]

            def one_matmul(ps, pt, h, col, step):
                (k0, k1), pm, first, last = step
                ksl = k0 if k1 == k0 + 1 and pm is None and not use_fp8 else None
                out = ps[:, h * FREE : (h + 1) * FREE]
                if use_fp8 and k1 == k0 + 1:
                    lhs = fT3[:, k0, pt * P : (pt + 1) * P]
                    rhs = mbT3[:, k0, col : col + FREE]
                elif k1 == k0 + 1:
                    lhs = fT3[:, k0, pt * P : (pt + 1) * P]
                    rhs = mbT3[:, k0, col : col + FREE]
                else:
                    lhs = fT3[:, k0:k1, pt * P : (pt + 1) * P]
                    rhs = mbT3[:, k0:k1, col : col + FREE]
                nc.tensor.matmul(out, lhs, rhs, start=first, stop=last, perf_mode=pm)

            def group_matmuls(ps, pt, bt):
                nh = gfree // FREE
                steps = half_steps()
                if use_ord:
                    # step-major: consecutive matmuls share the stationary lhsT
                    for step in steps:
                        for h in range(nh):
                            one_matmul(ps, pt, h, bt * gfree + h * FREE, step)
                else:
                    for h in range(nh):
                        for step in steps:
                            one_matmul(ps, pt, h, bt * gfree + h * FREE, step)

            use_split = os.environ.get("KBENCH_SPLIT", "1") != "0"
            split_bt_outer = os.environ.get("KBENCH_SPLIT", "1") == "2"
            chunk = int(os.environ.get("KBENCH_CHUNK", "0"))
            nsplit = int(os.environ.get("KBENCH_NSPLIT", "2"))
            if use_split:
                # pt-outer with j-slice ordering: tiles j < 8/nsplit of every
                # image complete first, so all but the last ReduceScatter run
                # while later matmul slices are still on the PE
                jw = (NPT // B) // nsplit
                gmax3 = persist.tile([P, B, NPT // B], dt.float32, name="gmax3")
                cc_outs = []
                for j_half in range(nsplit):
                    half_pts = [
                        (NPT // B * b_ + j, b_, j)
                        for j in range(jw * j_half, jw * (j_half + 1))
                        for b_ in range(B)
                    ]
                    if split_bt_outer:
                        for bt in range(nbt):
                            for pt, b_, j in half_pts:
                                ps = psum_mm.tile(
                                    [P, gfree], dt.float32, name="ps", tag="ps"
                                )
                                group_matmuls(ps, pt, bt)
                                nc.vector.reduce_max(
                                    maxall[:, pt, bt : bt + 1], ps[:], AX.X
                                )
                        for pt, b_, j in half_pts:
                            nc.vector.reduce_max(
                                gmax3[:, b_, j : j + 1], maxall[:, pt, :], AX.X
                            )
                    else:
                        j0 = jw * j_half
                        for b_ in range(B):
                            for j in range(j0, j0 + jw):
                                pt = NPT // B * b_ + j
                                for bt in range(nbt):
                                    ps = psum_mm.tile(
                                        [P, gfree], dt.float32, name="ps", tag="ps"
                                    )
                                    group_matmuls(ps, pt, bt)
                                    nc.vector.reduce_max(
                                        maxall[:, pt, bt : bt + 1], ps[:], AX.X
                                    )
                            pt0 = NPT // B * b_ + j0
                            nc.vector.reduce_max(
                                gmax3[:, b_, j0 : j0 + jw],
                                maxall[:, pt0 : pt0 + jw, :],
                                AX.X,
                            )
                    gh3 = io.tile([P, B, jw], dt.float32, name="gh3", tag="gh3")
                    nc.scalar.copy(
                        gh3[:], gmax3[:, :, jw * j_half : jw * (j_half + 1)]
                    )
                    tgh = psum_tp.tile([jw * B, P], dt.float32, name="tgh", tag="tp")
                    nc.tensor.transpose(tgh[:], gh3[:], ident_f[:])
                    ccs = io.tile([jw * B, P], dt.float32, name="ccs", tag="ccs")
                    nc.scalar.copy(ccs[:], tgh[:])
                    cc_in_h = dram.tile([jw * B, P], dt.float32, name=f"cc_in{j_half}")
                    nc.sync.dma_start(cc_in_h[:], ccs[:])
                    cc_out_h = dram.tile([jw, P], dt.float32, name=f"cc_out{j_half}")
                    nc.gpsimd.collective_compute(
                        "ReduceScatter",
                        ALU.max,
                        replica_groups=[list(range(NCORES))],
                        ins=[cc_in_h[:]],
                        outs=[cc_out_h[:]],
                    )
                    cc_outs.append(cc_out_h)
            elif chunk > 0:
                # stationary-reuse order: for each patch tile, sweep `chunk`
                # bank groups per k-step so consecutive matmuls share lhsT
                # (LDWEIGHTS amortized over chunk * halves matmuls)
                steps = half_steps()
                nh = gfree // FREE
                for pt in range(NPT):
                    for bt0 in range(0, nbt, chunk):
                        bts = list(range(bt0, min(bt0 + chunk, nbt)))
                        pss = [
                            psum_mm.tile([P, gfree], dt.float32, name="ps", tag="ps")
                            for _ in bts
                        ]
                        for step in steps:
                            for ps, bt in zip(pss, bts):
                                for h in range(nh):
                                    one_matmul(
                                        ps, pt, h, bt * gfree + h * FREE, step
                                    )
                        for ps, bt in zip(pss, bts):
                            nc.vector.reduce_max(
                                maxall[:, pt, bt : bt + 1], ps[:], AX.X
                            )
            else:
                for bt in range(nbt):
                    for pt in range(NPT):
                        ps = psum_mm.tile([P, gfree], dt.float32, name="ps", tag="ps")
                        group_matmuls(ps, pt, bt)
                        if os.environ.get("KBENCH_HALFRED") == "1":
                            nc.vector.reduce_max(
                                maxall[:, pt, bt : bt + 1], ps[:, 0:FREE], AX.X
                            )
                        else:
                            nc.vector.reduce_max(
                                maxall[:, pt, bt : bt + 1], ps[:], AX.X
                            )

            use_ag = os.environ.get("KBENCH_AG", "0") == "1"
            s32 = io.tile([GRID, GRID], dt.float32, name="s32", tag="s32")
            s1k_src = None
            if use_split:
                pass
            else:
                nc.vector.reduce_max(gmax[:], maxall[:], AX.X)
                # pack per-patch maxima in patch order: [128,64]->[64,128]
                tg = psum_tp.tile([NPT, P], dt.float32, name="tg", tag="tp")
                nc.tensor.transpose(tg[:], gmax[:], ident_f[:])
                cc_sb = io.tile([NPT, P], dt.float32, name="cc_sb", tag="cc_sb")
                nc.scalar.copy(cc_sb[:], tg[:])
                cc_in = dram.tile([NPT, P], dt.float32, name="cc_in")
                nc.sync.dma_start(cc_in[:], cc_sb[:])
            if use_split:
                rows = GRID // nsplit
                for q, cc_out_h in enumerate(cc_outs):
                    nc.sync.dma_start(s32[q * rows : (q + 1) * rows, :], cc_out_h[:])
            elif use_ag:
                # AllGather all 8 partial-max vectors, reduce + select locally
                ag_out = dram.tile(
                    [NCORES * NPT, P], dt.float32, name="ag_out", addr_space="Shared"
                )
                nc.gpsimd.collective_compute(
                    "AllGather",
                    ALU.bypass,
                    replica_groups=[list(range(NCORES))],
                    ins=[cc_in[:]],
                    outs=[ag_out[:]],
                )
                agt = io.tile([NPT, NCORES, P], dt.float32, name="agt", tag="agt")
                nc.sync.dma_start(
                    agt[:], ag_out[:].rearrange("(b r) j -> r b j", r=NPT)
                )
                g = io.tile([NPT, P], dt.float32, name="g", tag="g")
                nc.vector.tensor_max(g[:], agt[:, 0, :], agt[:, 1, :])
                for b_ in range(2, NCORES):
                    nc.vector.tensor_max(g[:], g[:], agt[:, b_, :])
                own_ps = psum_tp.tile([NCORES, P], dt.float32, name="own_ps", tag="tp")
                nc.tensor.matmul(own_ps[:], sel_sb[:], g[:], start=True, stop=True)
                own_sb = io.tile([NCORES, P], dt.float32, name="own_sb", tag="own_sb")
                nc.scalar.copy(own_sb[:], own_ps[:])
                nc.sync.dma_start(s32[:], own_sb[:])
                s1k_src = own_sb
            else:
                cc_out = dram.tile([GRID, GRID], dt.float32, name="cc_out")
                if os.environ.get("KBENCH_NOCC"):
                    nc.sync.dma_start(cc_out[:], cc_in[0:8, :])
                else:
                    nc.gpsimd.collective_compute(
                        "ReduceScatter",
                        ALU.max,
                        replica_groups=[list(range(NCORES))],
                        ins=[cc_in[:]],
                        outs=[cc_out[:]],
                    )
                nc.sync.dma_start(s32[:], cc_out[:])

            # core c now holds max-sim for its own image's 1024 patches
            d32 = io.tile([GRID, GRID], dt.float32, name="d32", tag="s32")
            nc.scalar.activation(d32[:], s32[:], AF.Copy, bias=1.0, scale=-1.0)
            nc.vector.tensor_scalar(d32[:], d32[:], 0.0, 2.0, ALU.max, ALU.min)

            s1k = io.tile([1, B * N // B], dt.float32, name="s1k", tag="s1k")
            if use_split:
                seg = (B * N // B) // nsplit
                for q, cc_out_h in enumerate(cc_outs):
                    nc.sync.dma_start(s1k[:, q * seg : (q + 1) * seg], cc_out_h[:])
            else:
                nc.sync.dma_start(s1k[:], s1k_src[:] if use_ag else cc_out[:])
            d1k = io.tile([1, B * N // B], dt.float32, name="d1k", tag="s1k")
            nc.scalar.activation(d1k[:], s1k[:], AF.Copy, bias=1.0, scale=-1.0)
            nc.vector.tensor_scalar(d1k[:], d1k[:], 0.0, 2.0, ALU.max, ALU.min)

            # pred_score: mean of top-10 = (sum(top8) + ranks 9,10) / 10
            top8 = stats.tile([1, 8], dt.float32, name="top8", tag="top8")
            nc.vector.max(top8[:], d1k[:])
            d1kr = io.tile([1, B * N // B], dt.float32, name="d1kr", tag="s1k")
            nc.vector.match_replace(d1kr[:], top8[:], d1k[:], -1e30)
            nxt8 = stats.tile([1, 8], dt.float32, name="nxt8", tag="top8")
            nc.vector.max(nxt8[:], d1kr[:])
            s8 = stats.tile([1, 1], dt.float32, name="s8", tag="acc")
            nc.vector.reduce_sum(s8[:], top8[:], AX.X)
            s2 = stats.tile([1, 1], dt.float32, name="s2", tag="acc")
            nc.vector.reduce_sum(s2[:], nxt8[:, 0:2], AX.X)
            ssum = stats.tile([1, 1], dt.float32, name="ssum", tag="acc")
            nc.vector.tensor_add(ssum[:], s8[:], s2[:])
            sc = stats.tile([1, 1], dt.float32, name="sc", tag="acc")
            nc.scalar.mul(sc[:], ssum[:], 1.0 / NUM_TOP)
            nc.sync.dma_start(score_ext[:], sc[:])

            # anomaly map: out = L @ A @ L^T, via AT then two matmuls with LT
            at_ps = psum_tp.tile([GRID, GRID], dt.float32, name="at_ps", tag="tp")
            nc.tensor.transpose(at_ps[:], d32[:], ident_f[0:GRID, 0:GRID])
            at = io.tile([GRID, GRID], dt.float32, name="at", tag="s32")
            nc.scalar.copy(at[:], at_ps[:])
            p_ps = psum_tp.tile([GRID, IMG], dt.float32, name="p_ps", tag="tp")
            nc.tensor.matmul(p_ps[:], at[:], lt_sb[:], start=True, stop=True)
            p_sb = io.tile([GRID, IMG], dt.float32, name="p_sb", tag="p_sb")
            nc.scalar.copy(p_sb[:], p_ps[:])
            for it in range(4):
                ow = min(P, IMG - it * P)
                o_ps = psum_tp.tile([P, IMG], dt.float32, name="o_ps", tag="tp")
                nc.tensor.matmul(
                    o_ps[0:ow, :],
                    lt_sb[:, it * P : it * P + ow],
                    p_sb[:],
                    start=True,
                    stop=True,
                )
                o_sb = io.tile([P, IMG], dt.float32, name="o_sb", tag="o_sb")
                nc.scalar.copy(o_sb[0:ow, :], o_ps[0:ow, :])
                nc.sync.dma_start(amap_ext[it * P : it * P + ow, :], o_sb[0:ow, :])

    nc.compile()
    return nc


def _in_maps(features: np.ndarray, memory_bank: np.ndarray):
    import os

    f = np.ascontiguousarray(np.asarray(features, dtype=np.float32).reshape(B * N, D))
    mb = np.asarray(memory_bank, dtype=np.float32)
    lt = _build_lt()
    in_maps = [
        {
            "features": f,
            "mb": np.ascontiguousarray(mb[c * MSH : (c + 1) * MSH]),
            "lt": lt,
        }
        for c in range(NCORES)
    ]
    if os.environ.get("KBENCH_AG", "0") == "1":
        for c in range(NCORES):
            sel = np.zeros((NPT, NCORES), dtype=np.float32)
            for i in range(NCORES):
                sel[NCORES * c + i, i] = 1.0
            in_maps[c]["sel"] = sel
    return in_maps


def _run(features: np.ndarray, memory_bank: np.ndarray, trace: bool = False):
    nc = _get_nc()
    from concourse.bass_utils import run_bass_kernel_spmd

    in_maps = _in_maps(features, memory_bank)
    res = run_bass_kernel_spmd(nc, in_maps, core_ids=list(range(NCORES)), trace=trace)
    results = res.results
    pred = np.stack(
        [results[c]["score"].reshape(1) for c in range(NCORES)], axis=0
    ).astype(np.float32)
    amap = np.stack([results[c]["amap"] for c in range(NCORES)], axis=0).reshape(
        B, 1, IMG, IMG
    ).astype(np.float32)
    return (pred, amap), res


def kernel(features: np.ndarray, memory_bank: np.ndarray):
    (pred, amap), _ = _run(features, memory_bank, trace=False)
    return pred, amap


# revision 45
# speedup vs baseline: 1.0798x; 1.0798x over previous
"""AnomalyDINO kNN retrieval kernel for one TRN2 chip (8 NeuronCores).

Strategy (sharded-kNN):
  - memory bank rows (M=65536) sharded 8-ways: each core owns 8192 rows
  - every core computes max-cosine-sim of ALL 8192 patches (B*N) against its
    bank shard: operands l2-normalized on device (f32 norms), cast to fp8
    e4m3, contraction padded 384->512 and run as 2 DoubleRow matmuls per
    512-wide output (f32 PSUM accumulate); VectorE reduce_max per
    [128,1024] PSUM group collects per-patch maxima
  - two ReduceScatter(max) collectives (16KB each) combine the 8 partial
    maxima and hand core c exactly the 1024 patches of image c (patch
    p=b*1024+n); the patch loop is ordered so grid-tiles j<4 of every image
    finish first, letting the first collective run under the second half of
    the matmul work (the halves are routed so RS chunk c = image c's half)
  - each core finishes its own image on-device: dist = clip(1-sim,0,2),
    pred_score = mean(top-10 of 1024) via Max8 + MatchReplace + Max8,
    anomaly map = L @ A(32x32) @ L^T (f32 TensorE matmuls) where
    L = (gaussian-blur-448 @ bilinear-resize-448x32) is a host-precomputed
    constant (exact linear operator for jax.image.resize + kornia-style
    reflect-pad blur, verified to 1.4e-6 against the jax reference)

fp8 e4m3 end-to-end rel err vs the f32 reference: 6.7e-3 (gate 2e-2).
"""

import functools
import sys

import numpy as np

B, N, D = 8, 1024, 384
M = 65536
NCORES = 8
MSH = M // NCORES        # bank rows per core
P = 128                  # SBUF partitions
NPT = (B * N) // P       # 64 patch tiles
FREE = 512               # matmul moving free dim (one PSUM bank of f32)
GFREE = 1024             # PSUM group width (2 banks, reduced by one DVE op)
NBT = MSH // GFREE       # 8 bank-tile groups per core
KC = D // P              # 3 contraction chunks
GRID = 32
IMG = 448
SIGMA = 4.0
NUM_TOP = 10


def _build_lt() -> np.ndarray:
    """LT = (Blur448 @ Resize448x32)^T as float32 [32, 448].

    Resize: jax.image.resize bilinear (half-pixel centers, clamped edges).
    Blur: sigma=4, radius=16 separable gaussian with reflect padding.
    """
    scale = GRID / IMG
    R = np.zeros((IMG, GRID), dtype=np.float64)
    for i in range(IMG):
        u = (i + 0.5) * scale - 0.5
        lo = int(np.floor(u))
        frac = u - lo
        for j, w in ((lo, 1.0 - frac), (lo + 1, frac)):
            R[i, min(max(j, 0), GRID - 1)] += w
    radius = int(4.0 * SIGMA + 0.5)
    t = np.arange(-radius, radius + 1, dtype=np.float64)
    k = np.exp(-0.5 * (t / SIGMA) ** 2)
    k /= k.sum()
    Bm = np.zeros((IMG, IMG), dtype=np.float64)
    for i in range(IMG):
        for tt in range(2 * radius + 1):
            j = i - radius + tt
            while j < 0 or j >= IMG:
                j = -j if j < 0 else 2 * (IMG - 1) - j
            Bm[i, j] += k[tt]
    return np.ascontiguousarray((Bm @ R).T.astype(np.float32))


@functools.lru_cache(maxsize=1)
def _get_nc():
    import os

    gfree = int(os.environ.get("KBENCH_GF", GFREE))
    nbt_all = MSH // gfree
    nbt = int(os.environ.get("KBENCH_NBT", nbt_all))
    nprep = int(os.environ.get("KBENCH_NPREP", 64))
    use_fp8 = os.environ.get("KBENCH_FP8", "1") == "1"
    use_ttr = os.environ.get("KBENCH_TTR", "0") == "1"
    use_ord = os.environ.get("KBENCH_ORD", "1") == "1"
    use_pad = os.environ.get("KBENCH_PAD", "1") == "1"
    chunk3 = int(os.environ.get("KBENCH_C3", "0"))
    kc = 4 if (use_fp8 and use_pad) else KC
    if "/opt/trn_rl_repo" not in sys.path:
        sys.path.insert(0, "/opt/trn_rl_repo")
    from concourse import bacc, masks, mybir, tile

    if os.environ.get("KBENCH_LDWOPT", "0") == "1":
        from concourse import bass_utils as _bu

        if not getattr(_bu, "_ldwopt_patched", False):
            _orig_run_command = _bu.run_command

            def _patched_run_command(argv, **kw):
                argv = [
                    "--enable-ldw-opt=true" if a == "--enable-ldw-opt=false" else a
                    for a in argv
                ]
                return _orig_run_command(argv, **kw)

            _bu.run_command = _patched_run_command
            _bu._ldwopt_patched = True

    dt = mybir.dt
    AX = mybir.AxisListType
    AF = mybir.ActivationFunctionType
    ALU = mybir.AluOpType

    nc = bacc.Bacc(
        "TRN2",
        target_bir_lowering=False,
        debug=False,
        enable_asserts=False,
        num_devices=NCORES,
    )

    f_ext = nc.dram_tensor("features", [B * N, D], dt.float32, kind="ExternalInput")
    mb_ext = nc.dram_tensor("mb", [MSH, D], dt.float32, kind="ExternalInput")
    lt_ext = nc.dram_tensor("lt", [GRID, IMG], dt.float32, kind="ExternalInput")
    amap_ext = nc.dram_tensor("amap", [IMG, IMG], dt.float32, kind="ExternalOutput")
    score_ext = nc.dram_tensor("score", [1, 1], dt.float32, kind="ExternalOutput")

    with tile.TileContext(nc) as tc:
        with (
            tc.tile_pool(name="persist", bufs=1) as persist,
            tc.tile_pool(name="io", bufs=4) as io,
            tc.tile_pool(name="stats", bufs=6) as stats,
            tc.tile_pool(
                name="psum_mm",
                bufs=(8 if chunk3 else 6) // (gfree // FREE),
                space="PSUM",
            ) as psum_mm,
            tc.tile_pool(name="psum_tp", bufs=2, space="PSUM") as psum_tp,
            tc.tile_pool(name="dram", bufs=1, space="DRAM") as dram,
        ):
            if os.environ.get("KBENCH_BAR", "0") == "1":
                # align cores at kernel entry so the later collective's
                # rendezvous wait overlaps the DMA/prep phase
                nc.gpsimd.bir_kernel_barrier_wait([list(range(NCORES))])
            ident_b = persist.tile([P, P], dt.bfloat16, name="ident_b")
            masks.make_identity(nc, ident_b[:])
            ident_f = persist.tile([P, P], dt.float32, name="ident_f")
            masks.make_identity(nc, ident_f[:])
            lt_sb = persist.tile([GRID, IMG], dt.float32, name="lt_sb")
            nc.sync.dma_start(lt_sb[:], lt_ext[:])
            if os.environ.get("KBENCH_AG", "0") == "1":
                sel_ext = nc.dram_tensor(
                    "sel", [NPT, NCORES], dt.float32, kind="ExternalInput"
                )
                sel_sb = persist.tile([NPT, NCORES], dt.float32, name="sel_sb")
                nc.sync.dma_start(sel_sb[:], sel_ext[:])

            op_dt = dt.float8e4 if use_fp8 else dt.bfloat16
            fT3 = persist.tile([P, kc, B * N], op_dt, name="fT3")
            mbT3 = persist.tile([P, kc, MSH], op_dt, name="mbT3")
            if kc == 4:
                nc.gpsimd.memset(fT3[:, 3, :], 0.0)
                nc.gpsimd.memset(mbT3[:, 3, :], 0.0)
            maxall = persist.tile([P, NPT, nbt_all], dt.float32, name="maxall")
            gmax = persist.tile([P, NPT], dt.float32, name="gmax")

            def tp_tile(shape, dtype, name):
                if chunk3:
                    return psum_mm.tile(shape, dtype, name=name, tag="ps")
                return psum_tp.tile(shape, dtype, name=name, tag="tp")

            prep_old = os.environ.get("KBENCH_PREPOLD") == "1"

            def prep(src, t, dstT):
                # load [128, 384] f32, l2-normalize rows, cast, transpose the
                # three 128x128 blocks onto the contraction axis
                ld = io.tile([P, D], dt.float32, name="ld", tag="ld")
                nc.sync.dma_start(ld[:], src[t * P : (t + 1) * P, :])
                sq = io.tile([P, D], dt.float32, name="sq", tag="sq")
                ss = stats.tile([P, 1], dt.float32, name="ss", tag="ss")
                nc.scalar.activation(sq[:], ld[:], AF.Square, accum_out=ss[:])
                nrm = stats.tile([P, 1], dt.float32, name="nrm", tag="nrm")
                nc.scalar.sqrt(nrm[:], ss[:])
                rin = stats.tile([P, 1], dt.float32, name="rin", tag="rin")
                nc.vector.reciprocal(rin[:], nrm[:])
                nb = io.tile([P, D], dt.bfloat16, name="nb", tag="nb")
                nc.scalar.activation(nb[:], ld[:], AF.Copy, scale=rin[:])
                for k in range(KC):
                    tp = psum_tp.tile([P, P], dt.bfloat16, name="tp", tag="tp")
                    nc.tensor.transpose(tp[:], nb[:, k * P : (k + 1) * P], ident_b[:])
                    nc.scalar.copy(dstT[:, k, t * P : (t + 1) * P], tp[:])

            PB = 4  # row-tiles per batched prep step

            def prep4(src_r, t4, dstT, eng):
                # batched prep: one DMA + fused normalize for PB row-tiles
                ld4 = io.tile([P, PB, D], dt.float32, name="ld4", tag="ld4")
                eng.dma_start(ld4[:], src_r[:, t4 * PB : (t4 + 1) * PB, :])
                sq4 = io.tile([P, PB, D], dt.float32, name="sq4", tag="sq4")
                nc.scalar.activation(sq4[:], ld4[:], AF.Square)
                ss4 = stats.tile([P, PB], dt.float32, name="ss4", tag="ss4")
                nc.vector.reduce_sum(ss4[:], sq4[:], AX.X)
                nrm4 = stats.tile([P, PB], dt.float32, name="nrm4", tag="nrm4")
                nc.scalar.sqrt(nrm4[:], ss4[:])
                rin4 = stats.tile([P, PB], dt.float32, name="rin4", tag="rin4")
                nc.vector.reciprocal(rin4[:], nrm4[:])
                nb4 = io.tile([P, PB, D], dt.bfloat16, name="nb4", tag="nb4")
                for j in range(PB):
                    nc.scalar.activation(
                        nb4[:, j, :], ld4[:, j, :], AF.Copy, scale=rin4[:, j : j + 1]
                    )
                for j in range(PB):
                    tp4 = tp_tile([P, KC, P], dt.bfloat16, "tp4")
                    for k in range(KC):
                        nc.tensor.transpose(
                            tp4[:, k, :], nb4[:, j, k * P : (k + 1) * P], ident_b[:]
                        )
                    t = t4 * PB + j
                    nc.scalar.copy(dstT[:, 0:KC, t * P : (t + 1) * P], tp4[:])

            if prep_old:
                for t in range(min(4, nprep)):
                    prep(mb_ext, t, mbT3)
                for t in range(nprep):
                    prep(f_ext, t, fT3)
                for t in range(4, nprep):
                    prep(mb_ext, t, mbT3)
            else:
                # rearranged [p, tile, d] views for batched loads
                f_r = f_ext[:].rearrange("(j p) d -> p j d", p=P)
                mb_r = mb_ext[:].rearrange("(j p) d -> p j d", p=P)
                nf4 = nprep // PB
                if os.environ.get("KBENCH_SPLIT", "1") == "2":
                    # bt-outer within half: first sweep needs mb tiles 0-7 and
                    # the even f groups
                    prep4(mb_r, 0, mbT3, nc.sync)
                    prep4(mb_r, 1, mbT3, nc.scalar)
                    for t4 in range(0, nf4, 2):
                        prep4(f_r, t4, fT3, nc.scalar if t4 % 2 else nc.sync)
                    for t4 in range(2, nf4):
                        prep4(mb_r, t4, mbT3, nc.scalar if t4 % 2 else nc.sync)
                    for t4 in range(1, nf4, 2):
                        prep4(f_r, t4, fT3, nc.scalar if t4 % 2 else nc.sync)
                elif os.environ.get("KBENCH_SPLIT", "1") != "0":
                    # split order consumes f tiles 0,8,16,.. first, and every
                    # sweep needs the whole bank: f(0), all mb, f evens, odds
                    prep4(f_r, 0, fT3, nc.sync)
                    for t4 in range(nf4):
                        prep4(mb_r, t4, mbT3, nc.scalar if t4 % 2 else nc.sync)
                    for t4 in list(range(2, nf4, 2)) + list(range(1, nf4, 2)):
                        prep4(f_r, t4, fT3, nc.scalar if t4 % 2 else nc.sync)
                else:
                    for t4 in range(min(1, nf4)):
                        prep4(mb_r, t4, mbT3, nc.sync)
                    for t4 in range(nf4):
                        prep4(f_r, t4, fT3, nc.scalar if t4 % 2 else nc.sync)
                    for t4 in range(1, nf4):
                        prep4(mb_r, t4, mbT3, nc.scalar if t4 % 2 else nc.sync)

            # main phase: (64 patch tiles) x (8 bank groups) x (2 halves);
            # fp8: k-chunks 0,1 as one DoubleRow matmul + chunk 2 plain (or
            # padded to 4 chunks = 2 DoubleRow matmuls)
            DR = mybir.MatmulPerfMode.DoubleRow

            def half_steps():
                # (k_slice, perf_mode, is_first, is_last) per accumulation step
                if not use_fp8:
                    return [
                        ((k, k + 1), None, k == 0, k == KC - 1) for k in range(KC)
                    ]
                if kc == 4:
                    return [((0, 2), DR, True, False), ((2, 4), DR, False, True)]
                return [((0, 2), DR, True, False), ((2, 3), None, False, True)]

            def one_matmul(ps, pt, h, col, step):
                (k0, k1), pm, first, last = step
                ksl = k0 if k1 == k0 + 1 and pm is None and not use_fp8 else None
                out = ps[:, h * FREE : (h + 1) * FREE]
                if use_fp8 and k1 == k0 + 1:
                    lhs = fT3[:, k0, pt * P : (pt + 1) * P]
                    rhs = mbT3[:, k0, col : col + FREE]
                elif k1 == k0 + 1:
                    lhs = fT3[:, k0, pt * P : (pt + 1) * P]
                    rhs = mbT3[:, k0, col : col + FREE]
                else:
                    lhs = fT3[:, k0:k1, pt * P : (pt + 1) * P]
                    rhs = mbT3[:, k0:k1, col : col + FREE]
                nc.tensor.matmul(out, lhs, rhs, start=first, stop=last, perf_mode=pm)

            def group_matmuls(ps, pt, bt):
                nh = gfree // FREE
                steps = half_steps()
                if use_ord:
                    # step-major: consecutive matmuls share the stationary lhsT
                    for step in steps:
                        for h in range(nh):
                            one_matmul(ps, pt, h, bt * gfree + h * FREE, step)
                else:
                    for h in range(nh):
                        for step in steps:
                            one_matmul(ps, pt, h, bt * gfree + h * FREE, step)

            use_split = os.environ.get("KBENCH_SPLIT", "1") != "0"
            split_bt_outer = os.environ.get("KBENCH_SPLIT", "1") == "2"
            chunk = int(os.environ.get("KBENCH_CHUNK", "0"))
            nsplit = int(os.environ.get("KBENCH_NSPLIT", "2"))
            if use_split:
                # pt-outer with j-slice ordering: tiles j < 8/nsplit of every
                # image complete first, so all but the last ReduceScatter run
                # while later matmul slices are still on the PE
                jw = (NPT // B) // nsplit
                gmax3 = persist.tile([P, B, NPT // B], dt.float32, name="gmax3")
                cc_outs = []
                for j_half in range(nsplit):
                    half_pts = [
                        (NPT // B * b_ + j, b_, j)
                        for j in range(jw * j_half, jw * (j_half + 1))
                        for b_ in range(B)
                    ]
                    if split_bt_outer:
                        for bt in range(nbt):
                            for pt, b_, j in half_pts:
                                ps = psum_mm.tile(
                                    [P, gfree], dt.float32, name="ps", tag="ps"
                                )
                                group_matmuls(ps, pt, bt)
                                nc.vector.reduce_max(
                                    maxall[:, pt, bt : bt + 1], ps[:], AX.X
                                )
                        for pt, b_, j in half_pts:
                            nc.vector.reduce_max(
                                gmax3[:, b_, j : j + 1], maxall[:, pt, :], AX.X
                            )
                    else:
                        j0 = jw * j_half
                        steps_l = half_steps()
                        for b_ in range(B):
                            for j in range(j0, j0 + jw):
                                pt = NPT // B * b_ + j
                                if chunk3:
                                    for bt0 in range(0, nbt, chunk3):
                                        bts = list(
                                            range(bt0, min(bt0 + chunk3, nbt))
                                        )
                                        pss = [
                                            psum_mm.tile(
                                                [P, gfree],
                                                dt.float32,
                                                name="ps",
                                                tag="ps",
                                            )
                                            for _ in bts
                                        ]
                                        for step in steps_l:
                                            for ps_, bt in zip(pss, bts):
                                                for h in range(gfree // FREE):
                                                    one_matmul(
                                                        ps_,
                                                        pt,
                                                        h,
                                                        bt * gfree + h * FREE,
                                                        step,
                                                    )
                                        for ps_, bt in zip(pss, bts):
                                            nc.vector.reduce_max(
                                                maxall[:, pt, bt : bt + 1],
                                                ps_[:],
                                                AX.X,
                                            )
                                else:
                                    for bt in range(nbt):
                                        ps = psum_mm.tile(
                                            [P, gfree], dt.float32, name="ps", tag="ps"
                                        )
                                        group_matmuls(ps, pt, bt)
                                        nc.vector.reduce_max(
                                            maxall[:, pt, bt : bt + 1], ps[:], AX.X
                                        )
                            pt0 = NPT // B * b_ + j0
                            nc.vector.reduce_max(
                                gmax3[:, b_, j0 : j0 + jw],
                                maxall[:, pt0 : pt0 + jw, :],
                                AX.X,
                            )
                    gh3 = io.tile([P, B, jw], dt.float32, name="gh3", tag="gh3")
                    nc.scalar.copy(
                        gh3[:], gmax3[:, :, jw * j_half : jw * (j_half + 1)]
                    )
                    tgh = tp_tile([jw * B, P], dt.float32, "tgh")
                    nc.tensor.transpose(tgh[:], gh3[:], ident_f[:])
                    ccs = io.tile([jw * B, P], dt.float32, name="ccs", tag="ccs")
                    nc.scalar.copy(ccs[:], tgh[:])
                    cc_in_h = dram.tile([jw * B, P], dt.float32, name=f"cc_in{j_half}")
                    nc.sync.dma_start(cc_in_h[:], ccs[:])
                    cc_out_h = dram.tile([jw, P], dt.float32, name=f"cc_out{j_half}")
                    nc.gpsimd.collective_compute(
                        "ReduceScatter",
                        ALU.max,
                        replica_groups=[list(range(NCORES))],
                        ins=[cc_in_h[:]],
                        outs=[cc_out_h[:]],
                    )
                    cc_outs.append(cc_out_h)
            elif chunk > 0:
                # stationary-reuse order: for each patch tile, sweep `chunk`
                # bank groups per k-step so consecutive matmuls share lhsT
                # (LDWEIGHTS amortized over chunk * halves matmuls)
                steps = half_steps()
                nh = gfree // FREE
                for pt in range(NPT):
                    for bt0 in range(0, nbt, chunk):
                        bts = list(range(bt0, min(bt0 + chunk, nbt)))
                        pss = [
                            psum_mm.tile([P, gfree], dt.float32, name="ps", tag="ps")
                            for _ in bts
                        ]
                        for step in steps:
                            for ps, bt in zip(pss, bts):
                                for h in range(nh):
                                    one_matmul(
                                        ps, pt, h, bt * gfree + h * FREE, step
                                    )
                        for ps, bt in zip(pss, bts):
                            nc.vector.reduce_max(
                                maxall[:, pt, bt : bt + 1], ps[:], AX.X
                            )
            else:
                for bt in range(nbt):
                    for pt in range(NPT):
                        ps = psum_mm.tile([P, gfree], dt.float32, name="ps", tag="ps")
                        group_matmuls(ps, pt, bt)
                        if os.environ.get("KBENCH_HALFRED") == "1":
                            nc.vector.reduce_max(
                                maxall[:, pt, bt : bt + 1], ps[:, 0:FREE], AX.X
                            )
                        else:
                            nc.vector.reduce_max(
                                maxall[:, pt, bt : bt + 1], ps[:], AX.X
                            )

            use_ag = os.environ.get("KBENCH_AG", "0") == "1"
            s32 = io.tile([GRID, GRID], dt.float32, name="s32", tag="s32")
            s1k_src = None
            if use_split:
                pass
            else:
                nc.vector.reduce_max(gmax[:], maxall[:], AX.X)
                # pack per-patch maxima in patch order: [128,64]->[64,128]
                tg = tp_tile([NPT, P], dt.float32, "tg")
                nc.tensor.transpose(tg[:], gmax[:], ident_f[:])
                cc_sb = io.tile([NPT, P], dt.float32, name="cc_sb", tag="cc_sb")
                nc.scalar.copy(cc_sb[:], tg[:])
                cc_in = dram.tile([NPT, P], dt.float32, name="cc_in")
                nc.sync.dma_start(cc_in[:], cc_sb[:])
            if use_split:
                rows = GRID // nsplit
                for q, cc_out_h in enumerate(cc_outs):
                    nc.sync.dma_start(s32[q * rows : (q + 1) * rows, :], cc_out_h[:])
            elif use_ag:
                # AllGather all 8 partial-max vectors, reduce + select locally
                ag_out = dram.tile(
                    [NCORES * NPT, P], dt.float32, name="ag_out", addr_space="Shared"
                )
                nc.gpsimd.collective_compute(
                    "AllGather",
                    ALU.bypass,
                    replica_groups=[list(range(NCORES))],
                    ins=[cc_in[:]],
                    outs=[ag_out[:]],
                )
                agt = io.tile([NPT, NCORES, P], dt.float32, name="agt", tag="agt")
                nc.sync.dma_start(
                    agt[:], ag_out[:].rearrange("(b r) j -> r b j", r=NPT)
                )
                g = io.tile([NPT, P], dt.float32, name="g", tag="g")
                nc.vector.tensor_max(g[:], agt[:, 0, :], agt[:, 1, :])
                for b_ in range(2, NCORES):
                    nc.vector.tensor_max(g[:], g[:], agt[:, b_, :])
                own_ps = psum_tp.tile([NCORES, P], dt.float32, name="own_ps", tag="tp")
                nc.tensor.matmul(own_ps[:], sel_sb[:], g[:], start=True, stop=True)
                own_sb = io.tile([NCORES, P], dt.float32, name="own_sb", tag="own_sb")
                nc.scalar.copy(own_sb[:], own_ps[:])
                nc.sync.dma_start(s32[:], own_sb[:])
                s1k_src = own_sb
            else:
                cc_out = dram.tile([GRID, GRID], dt.float32, name="cc_out")
                if os.environ.get("KBENCH_NOCC"):
                    nc.sync.dma_start(cc_out[:], cc_in[0:8, :])
                else:
                    nc.gpsimd.collective_compute(
                        "ReduceScatter",
                        ALU.max,
                        replica_groups=[list(range(NCORES))],
                        ins=[cc_in[:]],
                        outs=[cc_out[:]],
                    )
                nc.sync.dma_start(s32[:], cc_out[:])

            # core c now holds max-sim for its own image's 1024 patches
            d32 = io.tile([GRID, GRID], dt.float32, name="d32", tag="s32")
            nc.scalar.activation(d32[:], s32[:], AF.Copy, bias=1.0, scale=-1.0)
            nc.vector.tensor_scalar(d32[:], d32[:], 0.0, 2.0, ALU.max, ALU.min)

            s1k = io.tile([1, B * N // B], dt.float32, name="s1k", tag="s1k")
            if use_split:
                seg = (B * N // B) // nsplit
                for q, cc_out_h in enumerate(cc_outs):
                    nc.sync.dma_start(s1k[:, q * seg : (q + 1) * seg], cc_out_h[:])
            else:
                nc.sync.dma_start(s1k[:], s1k_src[:] if use_ag else cc_out[:])
            d1k = io.tile([1, B * N // B], dt.float32, name="d1k", tag="s1k")
            nc.scalar.activation(d1k[:], s1k[:], AF.Copy, bias=1.0, scale=-1.0)
            nc.vector.tensor_scalar(d1k[:], d1k[:], 0.0, 2.0, ALU.max, ALU.min)

            # pred_score: mean of top-10 = (sum(top8) + ranks 9,10) / 10
            top8 = stats.tile([1, 8], dt.float32, name="top8", tag="top8")
            nc.vector.max(top8[:], d1k[:])
            d1kr = io.tile([1, B * N // B], dt.float32, name="d1kr", tag="s1k")
            nc.vector.match_replace(d1kr[:], top8[:], d1k[:], -1e30)
            nxt8 = stats.tile([1, 8], dt.float32, name="nxt8", tag="top8")
            nc.vector.max(nxt8[:], d1kr[:])
            s8 = stats.tile([1, 1], dt.float32, name="s8", tag="acc")
            nc.vector.reduce_sum(s8[:], top8[:], AX.X)
            s2 = stats.tile([1, 1], dt.float32, name="s2", tag="acc")
            nc.vector.reduce_sum(s2[:], nxt8[:, 0:2], AX.X)
            ssum = stats.tile([1, 1], dt.float32, name="ssum", tag="acc")
            nc.vector.tensor_add(ssum[:], s8[:], s2[:])
            sc = stats.tile([1, 1], dt.float32, name="sc", tag="acc")
            nc.scalar.mul(sc[:], ssum[:], 1.0 / NUM_TOP)
            nc.sync.dma_start(score_ext[:], sc[:])

            # anomaly map: out = L @ A @ L^T, via AT then two matmuls with LT
            at_ps = tp_tile([GRID, GRID], dt.float32, "at_ps")
            nc.tensor.transpose(at_ps[:], d32[:], ident_f[0:GRID, 0:GRID])
            at = io.tile([GRID, GRID], dt.float32, name="at", tag="s32")
            nc.scalar.copy(at[:], at_ps[:])
            p_ps = tp_tile([GRID, IMG], dt.float32, "p_ps")
            nc.tensor.matmul(p_ps[:], at[:], lt_sb[:], start=True, stop=True)
            p_sb = io.tile([GRID, IMG], dt.float32, name="p_sb", tag="p_sb")
            nc.scalar.copy(p_sb[:], p_ps[:])
            for it in range(4):
                ow = min(P, IMG - it * P)
                o_ps = tp_tile([P, IMG], dt.float32, "o_ps")
                nc.tensor.matmul(
                    o_ps[0:ow, :],
                    lt_sb[:, it * P : it * P + ow],
                    p_sb[:],
                    start=True,
                    stop=True,
                )
                o_sb = io.tile([P, IMG], dt.float32, name="o_sb", tag="o_sb")
                nc.scalar.copy(o_sb[0:ow, :], o_ps[0:ow, :])
                nc.sync.dma_start(amap_ext[it * P : it * P + ow, :], o_sb[0:ow, :])

    nc.compile()
    return nc


def _in_maps(features: np.ndarray, memory_bank: np.ndarray):
    import os

    f = np.ascontiguousarray(np.asarray(features, dtype=np.float32).reshape(B * N, D))
    mb = np.asarray(memory_bank, dtype=np.float32)
    lt = _build_lt()
    in_maps = [
        {
            "features": f,
            "mb": np.ascontiguousarray(mb[c * MSH : (c + 1) * MSH]),
            "lt": lt,
        }
        for c in range(NCORES)
    ]
    if os.environ.get("KBENCH_AG", "0") == "1":
        for c in range(NCORES):
            sel = np.zeros((NPT, NCORES), dtype=np.float32)
            for i in range(NCORES):
                sel[NCORES * c + i, i] = 1.0
            in_maps[c]["sel"] = sel
    return in_maps


def _run(features: np.ndarray, memory_bank: np.ndarray, trace: bool = False):
    nc = _get_nc()
    from concourse.bass_utils import run_bass_kernel_spmd

    in_maps = _in_maps(features, memory_bank)
    res = run_bass_kernel_spmd(nc, in_maps, core_ids=list(range(NCORES)), trace=trace)
    results = res.results
    pred = np.stack(
        [results[c]["score"].reshape(1) for c in range(NCORES)], axis=0
    ).astype(np.float32)
    amap = np.stack([results[c]["amap"] for c in range(NCORES)], axis=0).reshape(
        B, 1, IMG, IMG
    ).astype(np.float32)
    return (pred, amap), res


def kernel(features: np.ndarray, memory_bank: np.ndarray):
    (pred, amap), _ = _run(features, memory_bank, trace=False)
    return pred, amap


# revision 47
# speedup vs baseline: 1.0807x; 1.0009x over previous
"""AnomalyDINO kNN retrieval kernel for one TRN2 chip (8 NeuronCores).

Strategy (sharded-kNN):
  - memory bank rows (M=65536) sharded 8-ways: each core owns 8192 rows
  - every core computes max-cosine-sim of ALL 8192 patches (B*N) against its
    bank shard: operands l2-normalized on device (f32 norms), cast to fp8
    e4m3, contraction padded 384->512 and run as 2 DoubleRow matmuls per
    512-wide output (f32 PSUM accumulate); VectorE reduce_max per
    [128,1024] PSUM group collects per-patch maxima
  - two ReduceScatter(max) collectives (16KB each) combine the 8 partial
    maxima and hand core c exactly the 1024 patches of image c (patch
    p=b*1024+n); the patch loop is ordered so grid-tiles j<4 of every image
    finish first, letting the first collective run under the second half of
    the matmul work (the halves are routed so RS chunk c = image c's half)
  - each core finishes its own image on-device: dist = clip(1-sim,0,2),
    pred_score = mean(top-10 of 1024) via Max8 + MatchReplace + Max8,
    anomaly map = L @ A(32x32) @ L^T (f32 TensorE matmuls) where
    L = (gaussian-blur-448 @ bilinear-resize-448x32) is a host-precomputed
    constant (exact linear operator for jax.image.resize + kornia-style
    reflect-pad blur, verified to 1.4e-6 against the jax reference)

fp8 e4m3 end-to-end rel err vs the f32 reference: 6.7e-3 (gate 2e-2).
"""

import functools
import sys

import numpy as np

B, N, D = 8, 1024, 384
M = 65536
NCORES = 8
MSH = M // NCORES        # bank rows per core
P = 128                  # SBUF partitions
NPT = (B * N) // P       # 64 patch tiles
FREE = 512               # matmul moving free dim (one PSUM bank of f32)
GFREE = 1024             # PSUM group width (2 banks, reduced by one DVE op)
NBT = MSH // GFREE       # 8 bank-tile groups per core
KC = D // P              # 3 contraction chunks
GRID = 32
IMG = 448
SIGMA = 4.0
NUM_TOP = 10


def _build_lt() -> np.ndarray:
    """LT = (Blur448 @ Resize448x32)^T as float32 [32, 448].

    Resize: jax.image.resize bilinear (half-pixel centers, clamped edges).
    Blur: sigma=4, radius=16 separable gaussian with reflect padding.
    """
    scale = GRID / IMG
    R = np.zeros((IMG, GRID), dtype=np.float64)
    for i in range(IMG):
        u = (i + 0.5) * scale - 0.5
        lo = int(np.floor(u))
        frac = u - lo
        for j, w in ((lo, 1.0 - frac), (lo + 1, frac)):
            R[i, min(max(j, 0), GRID - 1)] += w
    radius = int(4.0 * SIGMA + 0.5)
    t = np.arange(-radius, radius + 1, dtype=np.float64)
    k = np.exp(-0.5 * (t / SIGMA) ** 2)
    k /= k.sum()
    Bm = np.zeros((IMG, IMG), dtype=np.float64)
    for i in range(IMG):
        for tt in range(2 * radius + 1):
            j = i - radius + tt
            while j < 0 or j >= IMG:
                j = -j if j < 0 else 2 * (IMG - 1) - j
            Bm[i, j] += k[tt]
    return np.ascontiguousarray((Bm @ R).T.astype(np.float32))


@functools.lru_cache(maxsize=1)
def _get_nc():
    import os

    gfree = int(os.environ.get("KBENCH_GF", GFREE))
    nbt_all = MSH // gfree
    nbt = int(os.environ.get("KBENCH_NBT", nbt_all))
    nprep = int(os.environ.get("KBENCH_NPREP", 64))
    use_fp8 = os.environ.get("KBENCH_FP8", "1") == "1"
    use_ttr = os.environ.get("KBENCH_TTR", "0") == "1"
    use_ord = os.environ.get("KBENCH_ORD", "1") == "1"
    use_pad = os.environ.get("KBENCH_PAD", "1") == "1"
    chunk3 = int(os.environ.get("KBENCH_C3", "0"))
    use_swi = os.environ.get("KBENCH_SWI", "1") == "1"  # needs fp8+pad
    kc = 4 if (use_fp8 and use_pad) else KC
    if "/opt/trn_rl_repo" not in sys.path:
        sys.path.insert(0, "/opt/trn_rl_repo")
    from concourse import bacc, masks, mybir, tile

    if os.environ.get("KBENCH_LDWOPT", "0") == "1":
        from concourse import bass_utils as _bu

        if not getattr(_bu, "_ldwopt_patched", False):
            _orig_run_command = _bu.run_command

            def _patched_run_command(argv, **kw):
                argv = [
                    "--enable-ldw-opt=true" if a == "--enable-ldw-opt=false" else a
                    for a in argv
                ]
                return _orig_run_command(argv, **kw)

            _bu.run_command = _patched_run_command
            _bu._ldwopt_patched = True

    dt = mybir.dt
    AX = mybir.AxisListType
    AF = mybir.ActivationFunctionType
    ALU = mybir.AluOpType

    nc = bacc.Bacc(
        "TRN2",
        target_bir_lowering=False,
        debug=False,
        enable_asserts=False,
        num_devices=NCORES,
    )

    f_ext = nc.dram_tensor("features", [B * N, D], dt.float32, kind="ExternalInput")
    mb_ext = nc.dram_tensor("mb", [MSH, D], dt.float32, kind="ExternalInput")
    lt_ext = nc.dram_tensor("lt", [GRID, IMG], dt.float32, kind="ExternalInput")
    amap_ext = nc.dram_tensor("amap", [IMG, IMG], dt.float32, kind="ExternalOutput")
    score_ext = nc.dram_tensor("score", [1, 1], dt.float32, kind="ExternalOutput")

    with tile.TileContext(nc) as tc:
        with (
            tc.tile_pool(name="persist", bufs=1) as persist,
            tc.tile_pool(name="io", bufs=4) as io,
            tc.tile_pool(name="stats", bufs=6) as stats,
            tc.tile_pool(
                name="psum_mm",
                bufs=(8 if chunk3 else 6) // (gfree // FREE),
                space="PSUM",
            ) as psum_mm,
            tc.tile_pool(name="psum_tp", bufs=2, space="PSUM") as psum_tp,
            tc.tile_pool(name="dram", bufs=1, space="DRAM") as dram,
        ):
            if os.environ.get("KBENCH_BAR", "0") == "1":
                # align cores at kernel entry so the later collective's
                # rendezvous wait overlaps the DMA/prep phase
                nc.gpsimd.bir_kernel_barrier_wait([list(range(NCORES))])
            ident_b = persist.tile([P, P], dt.bfloat16, name="ident_b")
            masks.make_identity(nc, ident_b[:])
            ident_f = persist.tile([P, P], dt.float32, name="ident_f")
            masks.make_identity(nc, ident_f[:])
            if use_swi:
                ident_rev = persist.tile([P, P], dt.bfloat16, name="ident_rev")
                nc.gpsimd.memset(ident_rev[:], 0.0)
                nc.gpsimd.affine_select(
                    out=ident_rev[:],
                    in_=ident_rev[:],
                    compare_op=ALU.not_equal,
                    fill=1.0,
                    base=-(P - 1),
                    pattern=[[1, P]],
                    channel_multiplier=1,
                )
            lt_sb = persist.tile([GRID, IMG], dt.float32, name="lt_sb")
            nc.sync.dma_start(lt_sb[:], lt_ext[:])
            if os.environ.get("KBENCH_AG", "0") == "1":
                sel_ext = nc.dram_tensor(
                    "sel", [NPT, NCORES], dt.float32, kind="ExternalInput"
                )
                sel_sb = persist.tile([NPT, NCORES], dt.float32, name="sel_sb")
                nc.sync.dma_start(sel_sb[:], sel_ext[:])

            op_dt = dt.float8e4 if use_fp8 else dt.bfloat16
            if use_swi:
                # stationary weights pre-interleaved for DoubleRowSwInterleave:
                # [p, pair q, patch tile, reversed row, i] with chunk3 zero
                fTW = persist.tile([P, 2, NPT, P, 2], dt.float8e4, name="fTW")
                nc.gpsimd.memset(fTW[:, 1, :, :, 1], 0.0)
            else:
                fT3 = persist.tile([P, kc, B * N], op_dt, name="fT3")
                if kc == 4:
                    nc.gpsimd.memset(fT3[:, 3, :], 0.0)
            mbT3 = persist.tile([P, kc, MSH], op_dt, name="mbT3")
            if kc == 4:
                nc.gpsimd.memset(mbT3[:, 3, :], 0.0)
            maxall = persist.tile([P, NPT, nbt_all], dt.float32, name="maxall")
            gmax = persist.tile([P, NPT], dt.float32, name="gmax")

            def tp_tile(shape, dtype, name):
                if chunk3:
                    return psum_mm.tile(shape, dtype, name=name, tag="ps")
                return psum_tp.tile(shape, dtype, name=name, tag="tp")

            prep_old = os.environ.get("KBENCH_PREPOLD") == "1"

            def prep(src, t, dstT):
                # load [128, 384] f32, l2-normalize rows, cast, transpose the
                # three 128x128 blocks onto the contraction axis
                ld = io.tile([P, D], dt.float32, name="ld", tag="ld")
                nc.sync.dma_start(ld[:], src[t * P : (t + 1) * P, :])
                sq = io.tile([P, D], dt.float32, name="sq", tag="sq")
                ss = stats.tile([P, 1], dt.float32, name="ss", tag="ss")
                nc.scalar.activation(sq[:], ld[:], AF.Square, accum_out=ss[:])
                nrm = stats.tile([P, 1], dt.float32, name="nrm", tag="nrm")
                nc.scalar.sqrt(nrm[:], ss[:])
                rin = stats.tile([P, 1], dt.float32, name="rin", tag="rin")
                nc.vector.reciprocal(rin[:], nrm[:])
                nb = io.tile([P, D], dt.bfloat16, name="nb", tag="nb")
                nc.scalar.activation(nb[:], ld[:], AF.Copy, scale=rin[:])
                for k in range(KC):
                    tp = psum_tp.tile([P, P], dt.bfloat16, name="tp", tag="tp")
                    nc.tensor.transpose(tp[:], nb[:, k * P : (k + 1) * P], ident_b[:])
                    nc.scalar.copy(dstT[:, k, t * P : (t + 1) * P], tp[:])

            PB = 4  # row-tiles per batched prep step

            def prep4(src_r, t4, dstT, eng):
                # batched prep: one DMA + fused normalize for PB row-tiles
                ld4 = io.tile([P, PB, D], dt.float32, name="ld4", tag="ld4")
                eng.dma_start(ld4[:], src_r[:, t4 * PB : (t4 + 1) * PB, :])
                sq4 = io.tile([P, PB, D], dt.float32, name="sq4", tag="sq4")
                nc.scalar.activation(sq4[:], ld4[:], AF.Square)
                ss4 = stats.tile([P, PB], dt.float32, name="ss4", tag="ss4")
                nc.vector.reduce_sum(ss4[:], sq4[:], AX.X)
                nrm4 = stats.tile([P, PB], dt.float32, name="nrm4", tag="nrm4")
                nc.scalar.sqrt(nrm4[:], ss4[:])
                rin4 = stats.tile([P, PB], dt.float32, name="rin4", tag="rin4")
                nc.vector.reciprocal(rin4[:], nrm4[:])
                nb4 = io.tile([P, PB, D], dt.bfloat16, name="nb4", tag="nb4")
                for j in range(PB):
                    nc.scalar.activation(
                        nb4[:, j, :], ld4[:, j, :], AF.Copy, scale=rin4[:, j : j + 1]
                    )
                is_f = use_swi and dstT is None
                for j in range(PB):
                    tp4 = tp_tile([P, KC, P], dt.bfloat16, "tp4")
                    for k in range(KC):
                        nc.tensor.transpose(
                            tp4[:, k, :],
                            nb4[:, j, k * P : (k + 1) * P],
                            ident_rev[:] if is_f else ident_b[:],
                        )
                    t = t4 * PB + j
                    if is_f:
                        for k in range(KC):
                            q, i = divmod(k, 2)
                            nc.scalar.copy(fTW[:, q, t, :, i], tp4[:, k, :])
                    else:
                        nc.scalar.copy(dstT[:, 0:KC, t * P : (t + 1) * P], tp4[:])

            if prep_old:
                for t in range(min(4, nprep)):
                    prep(mb_ext, t, mbT3)
                for t in range(nprep):
                    prep(f_ext, t, fT3)
                for t in range(4, nprep):
                    prep(mb_ext, t, mbT3)
            else:
                # rearranged [p, tile, d] views for batched loads
                f_r = f_ext[:].rearrange("(j p) d -> p j d", p=P)
                mb_r = mb_ext[:].rearrange("(j p) d -> p j d", p=P)
                nf4 = nprep // PB
                if os.environ.get("KBENCH_SPLIT", "1") == "2":
                    # bt-outer within half: first sweep needs mb tiles 0-7 and
                    # the even f groups
                    prep4(mb_r, 0, mbT3, nc.sync)
                    prep4(mb_r, 1, mbT3, nc.scalar)
                    for t4 in range(0, nf4, 2):
                        prep4(f_r, t4, fT3, nc.scalar if t4 % 2 else nc.sync)
                    for t4 in range(2, nf4):
                        prep4(mb_r, t4, mbT3, nc.scalar if t4 % 2 else nc.sync)
                    for t4 in range(1, nf4, 2):
                        prep4(f_r, t4, None if use_swi else fT3, nc.scalar if t4 % 2 else nc.sync)
                elif os.environ.get("KBENCH_SPLIT", "1") != "0":
                    # split order consumes f tiles 0,8,16,.. first, and every
                    # sweep needs the whole bank: f(0), all mb, f evens, odds
                    prep4(f_r, 0, None if use_swi else fT3, nc.sync)
                    for t4 in range(nf4):
                        prep4(mb_r, t4, mbT3, nc.scalar if t4 % 2 else nc.sync)
                    for t4 in list(range(2, nf4, 2)) + list(range(1, nf4, 2)):
                        prep4(f_r, t4, None if use_swi else fT3, nc.scalar if t4 % 2 else nc.sync)
                else:
                    for t4 in range(min(1, nf4)):
                        prep4(mb_r, t4, mbT3, nc.sync)
                    for t4 in range(nf4):
                        prep4(f_r, t4, fT3, nc.scalar if t4 % 2 else nc.sync)
                    for t4 in range(1, nf4):
                        prep4(mb_r, t4, mbT3, nc.scalar if t4 % 2 else nc.sync)

            # main phase: (64 patch tiles) x (8 bank groups) x (2 halves);
            # fp8: k-chunks 0,1 as one DoubleRow matmul + chunk 2 plain (or
            # padded to 4 chunks = 2 DoubleRow matmuls)
            DR = mybir.MatmulPerfMode.DoubleRow

            def half_steps():
                # (k_slice, perf_mode, is_first, is_last) per accumulation step
                if not use_fp8:
                    return [
                        ((k, k + 1), None, k == 0, k == KC - 1) for k in range(KC)
                    ]
                if kc == 4:
                    return [((0, 2), DR, True, False), ((2, 4), DR, False, True)]
                return [((0, 2), DR, True, False), ((2, 3), None, False, True)]

            def one_matmul(ps, pt, h, col, step):
                (k0, k1), pm, first, last = step
                ksl = k0 if k1 == k0 + 1 and pm is None and not use_fp8 else None
                out = ps[:, h * FREE : (h + 1) * FREE]
                if use_fp8 and k1 == k0 + 1:
                    lhs = fT3[:, k0, pt * P : (pt + 1) * P]
                    rhs = mbT3[:, k0, col : col + FREE]
                elif k1 == k0 + 1:
                    lhs = fT3[:, k0, pt * P : (pt + 1) * P]
                    rhs = mbT3[:, k0, col : col + FREE]
                else:
                    lhs = fT3[:, k0:k1, pt * P : (pt + 1) * P]
                    rhs = mbT3[:, k0:k1, col : col + FREE]
                nc.tensor.matmul(out, lhs, rhs, start=first, stop=last, perf_mode=pm)

            SWI = mybir.MatmulPerfMode.DoubleRowSwInterleave

            def group_matmuls(ps, pt, bt):
                nh = gfree // FREE
                if use_swi:
                    for h in range(nh):
                        col = bt * gfree + h * FREE
                        out = ps[:, h * FREE : (h + 1) * FREE]
                        for q in range(2):
                            nc.tensor.matmul(
                                out,
                                fTW[:, q, pt, :, :],
                                mbT3[:, 2 * q : 2 * q + 2, col : col + FREE],
                                start=(q == 0),
                                stop=(q == 1),
                                perf_mode=SWI,
                            )
                    return
                steps = half_steps()
                if use_ord:
                    # step-major: consecutive matmuls share the stationary lhsT
                    for step in steps:
                        for h in range(nh):
                            one_matmul(ps, pt, h, bt * gfree + h * FREE, step)
                else:
                    for h in range(nh):
                        for step in steps:
                            one_matmul(ps, pt, h, bt * gfree + h * FREE, step)

            use_split = os.environ.get("KBENCH_SPLIT", "1") != "0"
            split_bt_outer = os.environ.get("KBENCH_SPLIT", "1") == "2"
            chunk = int(os.environ.get("KBENCH_CHUNK", "0"))
            nsplit = int(os.environ.get("KBENCH_NSPLIT", "2"))
            if use_split:
                # pt-outer with j-slice ordering: tiles j < 8/nsplit of every
                # image complete first, so all but the last ReduceScatter run
                # while later matmul slices are still on the PE
                jw = (NPT // B) // nsplit
                gmax3 = persist.tile([P, B, NPT // B], dt.float32, name="gmax3")
                cc_outs = []
                for j_half in range(nsplit):
                    half_pts = [
                        (NPT // B * b_ + j, b_, j)
                        for j in range(jw * j_half, jw * (j_half + 1))
                        for b_ in range(B)
                    ]
                    if split_bt_outer:
                        for bt in range(nbt):
                            for pt, b_, j in half_pts:
                                ps = psum_mm.tile(
                                    [P, gfree], dt.float32, name="ps", tag="ps"
                                )
                                group_matmuls(ps, pt, bt)
                                nc.vector.reduce_max(
                                    maxall[:, pt, bt : bt + 1], ps[:], AX.X
                                )
                        for pt, b_, j in half_pts:
                            nc.vector.reduce_max(
                                gmax3[:, b_, j : j + 1], maxall[:, pt, :], AX.X
                            )
                    else:
                        j0 = jw * j_half
                        steps_l = half_steps()
                        for b_ in range(B):
                            for j in range(j0, j0 + jw):
                                pt = NPT // B * b_ + j
                                if chunk3:
                                    for bt0 in range(0, nbt, chunk3):
                                        bts = list(
                                            range(bt0, min(bt0 + chunk3, nbt))
                                        )
                                        pss = [
                                            psum_mm.tile(
                                                [P, gfree],
                                                dt.float32,
                                                name="ps",
                                                tag="ps",
                                            )
                                            for _ in bts
                                        ]
                                        for step in steps_l:
                                            for ps_, bt in zip(pss, bts):
                                                for h in range(gfree // FREE):
                                                    one_matmul(
                                                        ps_,
                                                        pt,
                                                        h,
                                                        bt * gfree + h * FREE,
                                                        step,
                                                    )
                                        for ps_, bt in zip(pss, bts):
                                            nc.vector.reduce_max(
                                                maxall[:, pt, bt : bt + 1],
                                                ps_[:],
                                                AX.X,
                                            )
                                else:
                                    for bt in range(nbt):
                                        ps = psum_mm.tile(
                                            [P, gfree], dt.float32, name="ps", tag="ps"
                                        )
                                        group_matmuls(ps, pt, bt)
                                        red_in = (
                                            ps[:, 0:FREE]
                                            if os.environ.get("KBENCH_HALFRED") == "1"
                                            else ps[:]
                                        )
                                        nc.vector.reduce_max(
                                            maxall[:, pt, bt : bt + 1], red_in, AX.X
                                        )
                            pt0 = NPT // B * b_ + j0
                            nc.vector.reduce_max(
                                gmax3[:, b_, j0 : j0 + jw],
                                maxall[:, pt0 : pt0 + jw, :],
                                AX.X,
                            )
                    gh3 = io.tile([P, B, jw], dt.float32, name="gh3", tag="gh3")
                    nc.scalar.copy(
                        gh3[:], gmax3[:, :, jw * j_half : jw * (j_half + 1)]
                    )
                    tgh = tp_tile([jw * B, P], dt.float32, "tgh")
                    nc.tensor.transpose(tgh[:], gh3[:], ident_f[:])
                    ccs = io.tile([jw * B, P], dt.float32, name="ccs", tag="ccs")
                    nc.scalar.copy(ccs[:], tgh[:])
                    cc_in_h = dram.tile([jw * B, P], dt.float32, name=f"cc_in{j_half}")
                    nc.sync.dma_start(cc_in_h[:], ccs[:])
                    cc_out_h = dram.tile([jw, P], dt.float32, name=f"cc_out{j_half}")
                    nc.gpsimd.collective_compute(
                        "ReduceScatter",
                        ALU.max,
                        replica_groups=[list(range(NCORES))],
                        ins=[cc_in_h[:]],
                        outs=[cc_out_h[:]],
                    )
                    cc_outs.append(cc_out_h)
            elif chunk > 0:
                # stationary-reuse order: for each patch tile, sweep `chunk`
                # bank groups per k-step so consecutive matmuls share lhsT
                # (LDWEIGHTS amortized over chunk * halves matmuls)
                steps = half_steps()
                nh = gfree // FREE
                for pt in range(NPT):
                    for bt0 in range(0, nbt, chunk):
                        bts = list(range(bt0, min(bt0 + chunk, nbt)))
                        pss = [
                            psum_mm.tile([P, gfree], dt.float32, name="ps", tag="ps")
                            for _ in bts
                        ]
                        for step in steps:
                            for ps, bt in zip(pss, bts):
                                for h in range(nh):
                                    one_matmul(
                                        ps, pt, h, bt * gfree + h * FREE, step
                                    )
                        for ps, bt in zip(pss, bts):
                            nc.vector.reduce_max(
                                maxall[:, pt, bt : bt + 1], ps[:], AX.X
                            )
            else:
                for bt in range(nbt):
                    for pt in range(NPT):
                        ps = psum_mm.tile([P, gfree], dt.float32, name="ps", tag="ps")
                        group_matmuls(ps, pt, bt)
                        if os.environ.get("KBENCH_HALFRED") == "1":
                            nc.vector.reduce_max(
                                maxall[:, pt, bt : bt + 1], ps[:, 0:FREE], AX.X
                            )
                        else:
                            nc.vector.reduce_max(
                                maxall[:, pt, bt : bt + 1], ps[:], AX.X
                            )

            use_ag = os.environ.get("KBENCH_AG", "0") == "1"
            s32 = io.tile([GRID, GRID], dt.float32, name="s32", tag="s32")
            s1k_src = None
            if use_split:
                pass
            else:
                nc.vector.reduce_max(gmax[:], maxall[:], AX.X)
                # pack per-patch maxima in patch order: [128,64]->[64,128]
                tg = tp_tile([NPT, P], dt.float32, "tg")
                nc.tensor.transpose(tg[:], gmax[:], ident_f[:])
                cc_sb = io.tile([NPT, P], dt.float32, name="cc_sb", tag="cc_sb")
                nc.scalar.copy(cc_sb[:], tg[:])
                cc_in = dram.tile([NPT, P], dt.float32, name="cc_in")
                nc.sync.dma_start(cc_in[:], cc_sb[:])
            if use_split:
                rows = GRID // nsplit
                for q, cc_out_h in enumerate(cc_outs):
                    nc.sync.dma_start(s32[q * rows : (q + 1) * rows, :], cc_out_h[:])
            elif use_ag:
                # AllGather all 8 partial-max vectors, reduce + select locally
                ag_out = dram.tile(
                    [NCORES * NPT, P], dt.float32, name="ag_out", addr_space="Shared"
                )
                nc.gpsimd.collective_compute(
                    "AllGather",
                    ALU.bypass,
                    replica_groups=[list(range(NCORES))],
                    ins=[cc_in[:]],
                    outs=[ag_out[:]],
                )
                agt = io.tile([NPT, NCORES, P], dt.float32, name="agt", tag="agt")
                nc.sync.dma_start(
                    agt[:], ag_out[:].rearrange("(b r) j -> r b j", r=NPT)
                )
                g = io.tile([NPT, P], dt.float32, name="g", tag="g")
                nc.vector.tensor_max(g[:], agt[:, 0, :], agt[:, 1, :])
                for b_ in range(2, NCORES):
                    nc.vector.tensor_max(g[:], g[:], agt[:, b_, :])
                own_ps = psum_tp.tile([NCORES, P], dt.float32, name="own_ps", tag="tp")
                nc.tensor.matmul(own_ps[:], sel_sb[:], g[:], start=True, stop=True)
                own_sb = io.tile([NCORES, P], dt.float32, name="own_sb", tag="own_sb")
                nc.scalar.copy(own_sb[:], own_ps[:])
                nc.sync.dma_start(s32[:], own_sb[:])
                s1k_src = own_sb
            else:
                cc_out = dram.tile([GRID, GRID], dt.float32, name="cc_out")
                if os.environ.get("KBENCH_NOCC"):
                    nc.sync.dma_start(cc_out[:], cc_in[0:8, :])
                else:
                    nc.gpsimd.collective_compute(
                        "ReduceScatter",
                        ALU.max,
                        replica_groups=[list(range(NCORES))],
                        ins=[cc_in[:]],
                        outs=[cc_out[:]],
                    )
                nc.sync.dma_start(s32[:], cc_out[:])

            # core c now holds max-sim for its own image's 1024 patches
            d32 = io.tile([GRID, GRID], dt.float32, name="d32", tag="s32")
            nc.scalar.activation(d32[:], s32[:], AF.Copy, bias=1.0, scale=-1.0)
            nc.vector.tensor_scalar(d32[:], d32[:], 0.0, 2.0, ALU.max, ALU.min)

            s1k = io.tile([1, B * N // B], dt.float32, name="s1k", tag="s1k")
            if use_split:
                seg = (B * N // B) // nsplit
                for q, cc_out_h in enumerate(cc_outs):
                    nc.sync.dma_start(s1k[:, q * seg : (q + 1) * seg], cc_out_h[:])
            else:
                nc.sync.dma_start(s1k[:], s1k_src[:] if use_ag else cc_out[:])
            d1k = io.tile([1, B * N // B], dt.float32, name="d1k", tag="s1k")
            nc.scalar.activation(d1k[:], s1k[:], AF.Copy, bias=1.0, scale=-1.0)
            nc.vector.tensor_scalar(d1k[:], d1k[:], 0.0, 2.0, ALU.max, ALU.min)

            # pred_score: mean of top-10 = (sum(top8) + ranks 9,10) / 10
            top8 = stats.tile([1, 8], dt.float32, name="top8", tag="top8")
            nc.vector.max(top8[:], d1k[:])
            d1kr = io.tile([1, B * N // B], dt.float32, name="d1kr", tag="s1k")
            nc.vector.match_replace(d1kr[:], top8[:], d1k[:], -1e30)
            nxt8 = stats.tile([1, 8], dt.float32, name="nxt8", tag="top8")
            nc.vector.max(nxt8[:], d1kr[:])
            s8 = stats.tile([1, 1], dt.float32, name="s8", tag="acc")
            nc.vector.reduce_sum(s8[:], top8[:], AX.X)
            s2 = stats.tile([1, 1], dt.float32, name="s2", tag="acc")
            nc.vector.reduce_sum(s2[:], nxt8[:, 0:2], AX.X)
            ssum = stats.tile([1, 1], dt.float32, name="ssum", tag="acc")
            nc.vector.tensor_add(ssum[:], s8[:], s2[:])
            sc = stats.tile([1, 1], dt.float32, name="sc", tag="acc")
            nc.scalar.mul(sc[:], ssum[:], 1.0 / NUM_TOP)
            nc.sync.dma_start(score_ext[:], sc[:])

            # anomaly map: out = L @ A @ L^T, via AT then two matmuls with LT
            at_ps = tp_tile([GRID, GRID], dt.float32, "at_ps")
            nc.tensor.transpose(at_ps[:], d32[:], ident_f[0:GRID, 0:GRID])
            at = io.tile([GRID, GRID], dt.float32, name="at", tag="s32")
            nc.scalar.copy(at[:], at_ps[:])
            p_ps = tp_tile([GRID, IMG], dt.float32, "p_ps")
            nc.tensor.matmul(p_ps[:], at[:], lt_sb[:], start=True, stop=True)
            p_sb = io.tile([GRID, IMG], dt.float32, name="p_sb", tag="p_sb")
            nc.scalar.copy(p_sb[:], p_ps[:])
            for it in range(4):
                ow = min(P, IMG - it * P)
                o_ps = tp_tile([P, IMG], dt.float32, "o_ps")
                nc.tensor.matmul(
                    o_ps[0:ow, :],
                    lt_sb[:, it * P : it * P + ow],
                    p_sb[:],
                    start=True,
                    stop=True,
                )
                o_sb = io.tile([P, IMG], dt.float32, name="o_sb", tag="o_sb")
                nc.scalar.copy(o_sb[0:ow, :], o_ps[0:ow, :])
                nc.sync.dma_start(amap_ext[it * P : it * P + ow, :], o_sb[0:ow, :])

    nc.compile()
    return nc


def _in_maps(features: np.ndarray, memory_bank: np.ndarray):
    import os

    f = np.ascontiguousarray(np.asarray(features, dtype=np.float32).reshape(B * N, D))
    mb = np.asarray(memory_bank, dtype=np.float32)
    lt = _build_lt()
    in_maps = [
        {
            "features": f,
            "mb": np.ascontiguousarray(mb[c * MSH : (c + 1) * MSH]),
            "lt": lt,
        }
        for c in range(NCORES)
    ]
    if os.environ.get("KBENCH_AG", "0") == "1":
        for c in range(NCORES):
            sel = np.zeros((NPT, NCORES), dtype=np.float32)
            for i in range(NCORES):
                sel[NCORES * c + i, i] = 1.0
            in_maps[c]["sel"] = sel
    return in_maps


def _run(features: np.ndarray, memory_bank: np.ndarray, trace: bool = False):
    nc = _get_nc()
    from concourse.bass_utils import run_bass_kernel_spmd

    in_maps = _in_maps(features, memory_bank)
    res = run_bass_kernel_spmd(nc, in_maps, core_ids=list(range(NCORES)), trace=trace)
    results = res.results
    pred = np.stack(
        [results[c]["score"].reshape(1) for c in range(NCORES)], axis=0
    ).astype(np.float32)
    amap = np.stack([results[c]["amap"] for c in range(NCORES)], axis=0).reshape(
        B, 1, IMG, IMG
    ).astype(np.float32)
    return (pred, amap), res


def kernel(features: np.ndarray, memory_bank: np.ndarray):
    (pred, amap), _ = _run(features, memory_bank, trace=False)
    return pred, amap


# revision 48
# speedup vs baseline: 1.2626x; 1.1683x over previous
"""AnomalyDINO kNN retrieval kernel for one TRN2 chip (8 NeuronCores).

Strategy (sharded-kNN):
  - memory bank rows (M=65536) sharded 8-ways: each core owns 8192 rows
  - every core computes max-cosine-sim of ALL 8192 patches (B*N) against its
    bank shard: operands l2-normalized on device (f32 norms), cast to fp8
    e4m3, contraction padded 384->512 and run as 2 DoubleRow matmuls per
    512-wide output (f32 PSUM accumulate); VectorE reduce_max per
    [128,1024] PSUM group collects per-patch maxima
  - two ReduceScatter(max) collectives (16KB each) combine the 8 partial
    maxima and hand core c exactly the 1024 patches of image c (patch
    p=b*1024+n); the patch loop is ordered so grid-tiles j<4 of every image
    finish first, letting the first collective run under the second half of
    the matmul work (the halves are routed so RS chunk c = image c's half)
  - each core finishes its own image on-device: dist = clip(1-sim,0,2),
    pred_score = mean(top-10 of 1024) via Max8 + MatchReplace + Max8,
    anomaly map = L @ A(32x32) @ L^T (f32 TensorE matmuls) where
    L = (gaussian-blur-448 @ bilinear-resize-448x32) is a host-precomputed
    constant (exact linear operator for jax.image.resize + kornia-style
    reflect-pad blur, verified to 1.4e-6 against the jax reference)

fp8 e4m3 end-to-end rel err vs the f32 reference: 6.7e-3 (gate 2e-2).
"""

import functools
import sys

import numpy as np

B, N, D = 8, 1024, 384
M = 65536
NCORES = 8
MSH = M // NCORES        # bank rows per core
P = 128                  # SBUF partitions
NPT = (B * N) // P       # 64 patch tiles
FREE = 512               # matmul moving free dim (one PSUM bank of f32)
GFREE = 1024             # PSUM group width (2 banks, reduced by one DVE op)
NBT = MSH // GFREE       # 8 bank-tile groups per core
KC = D // P              # 3 contraction chunks
GRID = 32
IMG = 448
SIGMA = 4.0
NUM_TOP = 10


def _build_lt() -> np.ndarray:
    """LT = (Blur448 @ Resize448x32)^T as float32 [32, 448].

    Resize: jax.image.resize bilinear (half-pixel centers, clamped edges).
    Blur: sigma=4, radius=16 separable gaussian with reflect padding.
    """
    scale = GRID / IMG
    R = np.zeros((IMG, GRID), dtype=np.float64)
    for i in range(IMG):
        u = (i + 0.5) * scale - 0.5
        lo = int(np.floor(u))
        frac = u - lo
        for j, w in ((lo, 1.0 - frac), (lo + 1, frac)):
            R[i, min(max(j, 0), GRID - 1)] += w
    radius = int(4.0 * SIGMA + 0.5)
    t = np.arange(-radius, radius + 1, dtype=np.float64)
    k = np.exp(-0.5 * (t / SIGMA) ** 2)
    k /= k.sum()
    Bm = np.zeros((IMG, IMG), dtype=np.float64)
    for i in range(IMG):
        for tt in range(2 * radius + 1):
            j = i - radius + tt
            while j < 0 or j >= IMG:
                j = -j if j < 0 else 2 * (IMG - 1) - j
            Bm[i, j] += k[tt]
    return np.ascontiguousarray((Bm @ R).T.astype(np.float32))


@functools.lru_cache(maxsize=1)
def _get_nc():
    import os

    gfree = int(os.environ.get("KBENCH_GF", GFREE))
    nbt_all = MSH // gfree
    nbt = int(os.environ.get("KBENCH_NBT", nbt_all))
    nprep = int(os.environ.get("KBENCH_NPREP", 64))
    use_fp8 = os.environ.get("KBENCH_FP8", "1") == "1"
    use_ttr = os.environ.get("KBENCH_TTR", "0") == "1"
    use_ord = os.environ.get("KBENCH_ORD", "1") == "1"
    use_pad = os.environ.get("KBENCH_PAD", "1") == "1"
    chunk3 = int(os.environ.get("KBENCH_C3", "0"))
    use_swi = os.environ.get("KBENCH_SWI", "1") == "1"  # needs fp8+pad
    kc = 4 if (use_fp8 and use_pad) else KC
    if "/opt/trn_rl_repo" not in sys.path:
        sys.path.insert(0, "/opt/trn_rl_repo")
    from concourse import bacc, masks, mybir, tile

    if os.environ.get("KBENCH_LDWOPT", "0") == "1":
        from concourse import bass_utils as _bu

        if not getattr(_bu, "_ldwopt_patched", False):
            _orig_run_command = _bu.run_command

            def _patched_run_command(argv, **kw):
                argv = [
                    "--enable-ldw-opt=true" if a == "--enable-ldw-opt=false" else a
                    for a in argv
                ]
                return _orig_run_command(argv, **kw)

            _bu.run_command = _patched_run_command
            _bu._ldwopt_patched = True

    dt = mybir.dt
    AX = mybir.AxisListType
    AF = mybir.ActivationFunctionType
    ALU = mybir.AluOpType

    nc = bacc.Bacc(
        "TRN2",
        target_bir_lowering=False,
        debug=False,
        enable_asserts=False,
        num_devices=NCORES,
    )

    f_ext = nc.dram_tensor("features", [B * N, D], dt.float32, kind="ExternalInput")
    mb_ext = nc.dram_tensor("mb", [MSH, D], dt.float32, kind="ExternalInput")
    lt_ext = nc.dram_tensor("lt", [GRID, IMG], dt.float32, kind="ExternalInput")
    amap_ext = nc.dram_tensor("amap", [IMG, IMG], dt.float32, kind="ExternalOutput")
    score_ext = nc.dram_tensor("score", [1, 1], dt.float32, kind="ExternalOutput")

    with tile.TileContext(nc) as tc:
        with (
            tc.tile_pool(name="persist", bufs=1) as persist,
            tc.tile_pool(name="io", bufs=4) as io,
            tc.tile_pool(name="stats", bufs=6) as stats,
            tc.tile_pool(
                name="psum_mm",
                bufs=(8 if chunk3 else 6) // (gfree // FREE),
                space="PSUM",
            ) as psum_mm,
            tc.tile_pool(name="psum_tp", bufs=2, space="PSUM") as psum_tp,
            tc.tile_pool(name="dram", bufs=1, space="DRAM") as dram,
        ):
            if os.environ.get("KBENCH_BAR", "0") == "1":
                # align cores at kernel entry so the later collective's
                # rendezvous wait overlaps the DMA/prep phase
                nc.gpsimd.bir_kernel_barrier_wait([list(range(NCORES))])
            ident_b = persist.tile([P, P], dt.bfloat16, name="ident_b")
            masks.make_identity(nc, ident_b[:])
            ident_f = persist.tile([P, P], dt.float32, name="ident_f")
            masks.make_identity(nc, ident_f[:])
            if use_swi:
                ident_rev = persist.tile([P, P], dt.bfloat16, name="ident_rev")
                nc.gpsimd.memset(ident_rev[:], 0.0)
                nc.gpsimd.affine_select(
                    out=ident_rev[:],
                    in_=ident_rev[:],
                    compare_op=ALU.not_equal,
                    fill=1.0,
                    base=-(P - 1),
                    pattern=[[1, P]],
                    channel_multiplier=1,
                )
            lt_sb = persist.tile([GRID, IMG], dt.float32, name="lt_sb")
            nc.sync.dma_start(lt_sb[:], lt_ext[:])
            if os.environ.get("KBENCH_AG", "0") == "1":
                sel_ext = nc.dram_tensor(
                    "sel", [NPT, NCORES], dt.float32, kind="ExternalInput"
                )
                sel_sb = persist.tile([NPT, NCORES], dt.float32, name="sel_sb")
                nc.sync.dma_start(sel_sb[:], sel_ext[:])

            op_dt = dt.float8e4 if use_fp8 else dt.bfloat16
            if use_swi:
                # stationary weights pre-interleaved for DoubleRowSwInterleave:
                # [p, pair q, patch tile, reversed row, i] with chunk3 zero
                fTW = persist.tile([P, 2, NPT, P, 2], dt.float8e4, name="fTW")
                nc.gpsimd.memset(fTW[:, 1, :, :, 1], 0.0)
            else:
                fT3 = persist.tile([P, kc, B * N], op_dt, name="fT3")
                if kc == 4:
                    nc.gpsimd.memset(fT3[:, 3, :], 0.0)
            mbT3 = persist.tile([P, kc, MSH], op_dt, name="mbT3")
            if kc == 4:
                nc.gpsimd.memset(mbT3[:, 3, :], 0.0)
            maxall = persist.tile([P, NPT, nbt_all], dt.float32, name="maxall")
            gmax = persist.tile([P, NPT], dt.float32, name="gmax")

            def tp_tile(shape, dtype, name):
                if chunk3:
                    return psum_mm.tile(shape, dtype, name=name, tag="ps")
                return psum_tp.tile(shape, dtype, name=name, tag="tp")

            prep_old = os.environ.get("KBENCH_PREPOLD") == "1"

            def prep(src, t, dstT):
                # load [128, 384] f32, l2-normalize rows, cast, transpose the
                # three 128x128 blocks onto the contraction axis
                ld = io.tile([P, D], dt.float32, name="ld", tag="ld")
                nc.sync.dma_start(ld[:], src[t * P : (t + 1) * P, :])
                sq = io.tile([P, D], dt.float32, name="sq", tag="sq")
                ss = stats.tile([P, 1], dt.float32, name="ss", tag="ss")
                nc.scalar.activation(sq[:], ld[:], AF.Square, accum_out=ss[:])
                nrm = stats.tile([P, 1], dt.float32, name="nrm", tag="nrm")
                nc.scalar.sqrt(nrm[:], ss[:])
                rin = stats.tile([P, 1], dt.float32, name="rin", tag="rin")
                nc.vector.reciprocal(rin[:], nrm[:])
                nb = io.tile([P, D], dt.bfloat16, name="nb", tag="nb")
                nc.scalar.activation(nb[:], ld[:], AF.Copy, scale=rin[:])
                for k in range(KC):
                    tp = psum_tp.tile([P, P], dt.bfloat16, name="tp", tag="tp")
                    nc.tensor.transpose(tp[:], nb[:, k * P : (k + 1) * P], ident_b[:])
                    nc.scalar.copy(dstT[:, k, t * P : (t + 1) * P], tp[:])

            PB = 4  # row-tiles per batched prep step

            def prep4(src_r, t4, dstT, eng):
                # batched prep: one DMA + fused normalize for PB row-tiles
                ld4 = io.tile([P, PB, D], dt.float32, name="ld4", tag="ld4")
                eng.dma_start(ld4[:], src_r[:, t4 * PB : (t4 + 1) * PB, :])
                sq4 = io.tile([P, PB, D], dt.float32, name="sq4", tag="sq4")
                nc.scalar.activation(sq4[:], ld4[:], AF.Square)
                ss4 = stats.tile([P, PB], dt.float32, name="ss4", tag="ss4")
                nc.vector.reduce_sum(ss4[:], sq4[:], AX.X)
                nrm4 = stats.tile([P, PB], dt.float32, name="nrm4", tag="nrm4")
                nc.scalar.sqrt(nrm4[:], ss4[:])
                rin4 = stats.tile([P, PB], dt.float32, name="rin4", tag="rin4")
                nc.vector.reciprocal(rin4[:], nrm4[:])
                nb4 = io.tile([P, PB, D], dt.bfloat16, name="nb4", tag="nb4")
                for j in range(PB):
                    nc.scalar.activation(
                        nb4[:, j, :], ld4[:, j, :], AF.Copy, scale=rin4[:, j : j + 1]
                    )
                is_f = use_swi and dstT is None
                for j in range(PB):
                    tp4 = tp_tile([P, KC, P], dt.bfloat16, "tp4")
                    for k in range(KC):
                        nc.tensor.transpose(
                            tp4[:, k, :],
                            nb4[:, j, k * P : (k + 1) * P],
                            ident_rev[:] if is_f else ident_b[:],
                        )
                    t = t4 * PB + j
                    if is_f:
                        for k in range(KC):
                            q, i = divmod(k, 2)
                            nc.scalar.copy(fTW[:, q, t, :, i], tp4[:, k, :])
                    else:
                        nc.scalar.copy(dstT[:, 0:KC, t * P : (t + 1) * P], tp4[:])

            if prep_old:
                for t in range(min(4, nprep)):
                    prep(mb_ext, t, mbT3)
                for t in range(nprep):
                    prep(f_ext, t, fT3)
                for t in range(4, nprep):
                    prep(mb_ext, t, mbT3)
            else:
                # rearranged [p, tile, d] views for batched loads
                f_r = f_ext[:].rearrange("(j p) d -> p j d", p=P)
                mb_r = mb_ext[:].rearrange("(j p) d -> p j d", p=P)
                nf4 = nprep // PB
                if os.environ.get("KBENCH_SPLIT", "1") == "2":
                    # bt-outer within half: first sweep needs mb tiles 0-7 and
                    # the even f groups
                    prep4(mb_r, 0, mbT3, nc.sync)
                    prep4(mb_r, 1, mbT3, nc.scalar)
                    for t4 in range(0, nf4, 2):
                        prep4(f_r, t4, fT3, nc.scalar if t4 % 2 else nc.sync)
                    for t4 in range(2, nf4):
                        prep4(mb_r, t4, mbT3, nc.scalar if t4 % 2 else nc.sync)
                    for t4 in range(1, nf4, 2):
                        prep4(f_r, t4, None if use_swi else fT3, nc.scalar if t4 % 2 else nc.sync)
                elif os.environ.get("KBENCH_SPLIT", "1") != "0":
                    # split order consumes f tiles 0,8,16,.. first, and every
                    # sweep needs the whole bank: f(0), all mb, f evens, odds
                    prep4(f_r, 0, None if use_swi else fT3, nc.sync)
                    for t4 in range(nf4):
                        prep4(mb_r, t4, mbT3, nc.scalar if t4 % 2 else nc.sync)
                    for t4 in list(range(2, nf4, 2)) + list(range(1, nf4, 2)):
                        prep4(f_r, t4, None if use_swi else fT3, nc.scalar if t4 % 2 else nc.sync)
                else:
                    for t4 in range(min(1, nf4)):
                        prep4(mb_r, t4, mbT3, nc.sync)
                    for t4 in range(nf4):
                        prep4(f_r, t4, fT3, nc.scalar if t4 % 2 else nc.sync)
                    for t4 in range(1, nf4):
                        prep4(mb_r, t4, mbT3, nc.scalar if t4 % 2 else nc.sync)

            # main phase: (64 patch tiles) x (8 bank groups) x (2 halves);
            # fp8: k-chunks 0,1 as one DoubleRow matmul + chunk 2 plain (or
            # padded to 4 chunks = 2 DoubleRow matmuls)
            DR = mybir.MatmulPerfMode.DoubleRow

            def half_steps():
                # (k_slice, perf_mode, is_first, is_last) per accumulation step
                if not use_fp8:
                    return [
                        ((k, k + 1), None, k == 0, k == KC - 1) for k in range(KC)
                    ]
                if kc == 4:
                    return [((0, 2), DR, True, False), ((2, 4), DR, False, True)]
                return [((0, 2), DR, True, False), ((2, 3), None, False, True)]

            def one_matmul(ps, pt, h, col, step):
                (k0, k1), pm, first, last = step
                ksl = k0 if k1 == k0 + 1 and pm is None and not use_fp8 else None
                out = ps[:, h * FREE : (h + 1) * FREE]
                if use_fp8 and k1 == k0 + 1:
                    lhs = fT3[:, k0, pt * P : (pt + 1) * P]
                    rhs = mbT3[:, k0, col : col + FREE]
                elif k1 == k0 + 1:
                    lhs = fT3[:, k0, pt * P : (pt + 1) * P]
                    rhs = mbT3[:, k0, col : col + FREE]
                else:
                    lhs = fT3[:, k0:k1, pt * P : (pt + 1) * P]
                    rhs = mbT3[:, k0:k1, col : col + FREE]
                nc.tensor.matmul(out, lhs, rhs, start=first, stop=last, perf_mode=pm)

            SWI = mybir.MatmulPerfMode.DoubleRowSwInterleave

            def group_matmuls(ps, pt, bt):
                nh = gfree // FREE
                if use_swi:
                    for h in range(nh):
                        col = bt * gfree + h * FREE
                        out = ps[:, h * FREE : (h + 1) * FREE]
                        for q in range(2):
                            nc.tensor.matmul(
                                out,
                                fTW[:, q, pt, :, :],
                                mbT3[:, 2 * q : 2 * q + 2, col : col + FREE],
                                start=(q == 0),
                                stop=(q == 1),
                                perf_mode=SWI,
                            )
                    return
                steps = half_steps()
                if use_ord:
                    # step-major: consecutive matmuls share the stationary lhsT
                    for step in steps:
                        for h in range(nh):
                            one_matmul(ps, pt, h, bt * gfree + h * FREE, step)
                else:
                    for h in range(nh):
                        for step in steps:
                            one_matmul(ps, pt, h, bt * gfree + h * FREE, step)

            use_split = os.environ.get("KBENCH_SPLIT", "1") != "0"
            split_bt_outer = os.environ.get("KBENCH_SPLIT", "1") == "2"
            chunk = int(os.environ.get("KBENCH_CHUNK", "0"))
            nsplit = int(os.environ.get("KBENCH_NSPLIT", "2"))
            if use_split:
                # pt-outer with j-slice ordering: tiles j < 8/nsplit of every
                # image complete first, so all but the last ReduceScatter run
                # while later matmul slices are still on the PE
                jw = (NPT // B) // nsplit
                gmax3 = persist.tile([P, B, NPT // B], dt.float32, name="gmax3")
                cc_outs = []
                for j_half in range(nsplit):
                    half_pts = [
                        (NPT // B * b_ + j, b_, j)
                        for j in range(jw * j_half, jw * (j_half + 1))
                        for b_ in range(B)
                    ]
                    if split_bt_outer:
                        for bt in range(nbt):
                            for pt, b_, j in half_pts:
                                ps = psum_mm.tile(
                                    [P, gfree], dt.float32, name="ps", tag="ps"
                                )
                                group_matmuls(ps, pt, bt)
                                nc.vector.reduce_max(
                                    maxall[:, pt, bt : bt + 1], ps[:], AX.X
                                )
                        for pt, b_, j in half_pts:
                            nc.vector.reduce_max(
                                gmax3[:, b_, j : j + 1], maxall[:, pt, :], AX.X
                            )
                    else:
                        j0 = jw * j_half
                        steps_l = half_steps()
                        for b_ in range(B):
                            for j in range(j0, j0 + jw):
                                pt = NPT // B * b_ + j
                                if chunk3:
                                    for bt0 in range(0, nbt, chunk3):
                                        bts = list(
                                            range(bt0, min(bt0 + chunk3, nbt))
                                        )
                                        pss = [
                                            psum_mm.tile(
                                                [P, gfree],
                                                dt.float32,
                                                name="ps",
                                                tag="ps",
                                            )
                                            for _ in bts
                                        ]
                                        if use_swi:
                                            for ps_, bt in zip(pss, bts):
                                                group_matmuls(ps_, pt, bt)
                                        else:
                                            for step in steps_l:
                                                for ps_, bt in zip(pss, bts):
                                                    for h in range(gfree // FREE):
                                                        one_matmul(
                                                            ps_,
                                                            pt,
                                                            h,
                                                            bt * gfree + h * FREE,
                                                            step,
                                                        )
                                        for ps_, bt in zip(pss, bts):
                                            nc.vector.reduce_max(
                                                maxall[:, pt, bt : bt + 1],
                                                ps_[:],
                                                AX.X,
                                            )
                                else:
                                    for bt in range(nbt):
                                        ps = psum_mm.tile(
                                            [P, gfree], dt.float32, name="ps", tag="ps"
                                        )
                                        group_matmuls(ps, pt, bt)
                                        red_in = (
                                            ps[:, 0:FREE]
                                            if os.environ.get("KBENCH_HALFRED") == "1"
                                            else ps[:]
                                        )
                                        nc.vector.reduce_max(
                                            maxall[:, pt, bt : bt + 1], red_in, AX.X
                                        )
                            pt0 = NPT // B * b_ + j0
                            nc.vector.reduce_max(
                                gmax3[:, b_, j0 : j0 + jw],
                                maxall[:, pt0 : pt0 + jw, :],
                                AX.X,
                            )
                    gh3 = io.tile([P, B, jw], dt.float32, name="gh3", tag="gh3")
                    nc.scalar.copy(
                        gh3[:], gmax3[:, :, jw * j_half : jw * (j_half + 1)]
                    )
                    tgh = tp_tile([jw * B, P], dt.float32, "tgh")
                    nc.tensor.transpose(tgh[:], gh3[:], ident_f[:])
                    ccs = io.tile([jw * B, P], dt.float32, name="ccs", tag="ccs")
                    nc.scalar.copy(ccs[:], tgh[:])
                    cc_in_h = dram.tile([jw * B, P], dt.float32, name=f"cc_in{j_half}")
                    nc.sync.dma_start(cc_in_h[:], ccs[:])
                    cc_out_h = dram.tile([jw, P], dt.float32, name=f"cc_out{j_half}")
                    nc.gpsimd.collective_compute(
                        "ReduceScatter",
                        ALU.max,
                        replica_groups=[list(range(NCORES))],
                        ins=[cc_in_h[:]],
                        outs=[cc_out_h[:]],
                    )
                    cc_outs.append(cc_out_h)
            elif chunk > 0:
                # stationary-reuse order: for each patch tile, sweep `chunk`
                # bank groups per k-step so consecutive matmuls share lhsT
                # (LDWEIGHTS amortized over chunk * halves matmuls)
                steps = half_steps()
                nh = gfree // FREE
                for pt in range(NPT):
                    for bt0 in range(0, nbt, chunk):
                        bts = list(range(bt0, min(bt0 + chunk, nbt)))
                        pss = [
                            psum_mm.tile([P, gfree], dt.float32, name="ps", tag="ps")
                            for _ in bts
                        ]
                        for step in steps:
                            for ps, bt in zip(pss, bts):
                                for h in range(nh):
                                    one_matmul(
                                        ps, pt, h, bt * gfree + h * FREE, step
                                    )
                        for ps, bt in zip(pss, bts):
                            nc.vector.reduce_max(
                                maxall[:, pt, bt : bt + 1], ps[:], AX.X
                            )
            else:
                for bt in range(nbt):
                    for pt in range(NPT):
                        ps = psum_mm.tile([P, gfree], dt.float32, name="ps", tag="ps")
                        group_matmuls(ps, pt, bt)
                        if os.environ.get("KBENCH_HALFRED") == "1":
                            nc.vector.reduce_max(
                                maxall[:, pt, bt : bt + 1], ps[:, 0:FREE], AX.X
                            )
                        else:
                            nc.vector.reduce_max(
                                maxall[:, pt, bt : bt + 1], ps[:], AX.X
                            )

            use_ag = os.environ.get("KBENCH_AG", "0") == "1"
            s32 = io.tile([GRID, GRID], dt.float32, name="s32", tag="s32")
            s1k_src = None
            if use_split:
                pass
            else:
                nc.vector.reduce_max(gmax[:], maxall[:], AX.X)
                # pack per-patch maxima in patch order: [128,64]->[64,128]
                tg = tp_tile([NPT, P], dt.float32, "tg")
                nc.tensor.transpose(tg[:], gmax[:], ident_f[:])
                cc_sb = io.tile([NPT, P], dt.float32, name="cc_sb", tag="cc_sb")
                nc.scalar.copy(cc_sb[:], tg[:])
                cc_in = dram.tile([NPT, P], dt.float32, name="cc_in")
                nc.sync.dma_start(cc_in[:], cc_sb[:])
            if use_split:
                rows = GRID // nsplit
                for q, cc_out_h in enumerate(cc_outs):
                    nc.sync.dma_start(s32[q * rows : (q + 1) * rows, :], cc_out_h[:])
            elif use_ag:
                # AllGather all 8 partial-max vectors, reduce + select locally
                ag_out = dram.tile(
                    [NCORES * NPT, P], dt.float32, name="ag_out", addr_space="Shared"
                )
                nc.gpsimd.collective_compute(
                    "AllGather",
                    ALU.bypass,
                    replica_groups=[list(range(NCORES))],
                    ins=[cc_in[:]],
                    outs=[ag_out[:]],
                )
                agt = io.tile([NPT, NCORES, P], dt.float32, name="agt", tag="agt")
                nc.sync.dma_start(
                    agt[:], ag_out[:].rearrange("(b r) j -> r b j", r=NPT)
                )
                g = io.tile([NPT, P], dt.float32, name="g", tag="g")
                nc.vector.tensor_max(g[:], agt[:, 0, :], agt[:, 1, :])
                for b_ in range(2, NCORES):
                    nc.vector.tensor_max(g[:], g[:], agt[:, b_, :])
                own_ps = psum_tp.tile([NCORES, P], dt.float32, name="own_ps", tag="tp")
                nc.tensor.matmul(own_ps[:], sel_sb[:], g[:], start=True, stop=True)
                own_sb = io.tile([NCORES, P], dt.float32, name="own_sb", tag="own_sb")
                nc.scalar.copy(own_sb[:], own_ps[:])
                nc.sync.dma_start(s32[:], own_sb[:])
                s1k_src = own_sb
            else:
                cc_out = dram.tile([GRID, GRID], dt.float32, name="cc_out")
                if os.environ.get("KBENCH_NOCC"):
                    nc.sync.dma_start(cc_out[:], cc_in[0:8, :])
                else:
                    nc.gpsimd.collective_compute(
                        "ReduceScatter",
                        ALU.max,
                        replica_groups=[list(range(NCORES))],
                        ins=[cc_in[:]],
                        outs=[cc_out[:]],
                    )
                nc.sync.dma_start(s32[:], cc_out[:])

            # core c now holds max-sim for its own image's 1024 patches
            d32 = io.tile([GRID, GRID], dt.float32, name="d32", tag="s32")
            nc.scalar.activation(d32[:], s32[:], AF.Copy, bias=1.0, scale=-1.0)
            nc.vector.tensor_scalar(d32[:], d32[:], 0.0, 2.0, ALU.max, ALU.min)

            s1k = io.tile([1, B * N // B], dt.float32, name="s1k", tag="s1k")
            if use_split:
                seg = (B * N // B) // nsplit
                for q, cc_out_h in enumerate(cc_outs):
                    nc.sync.dma_start(s1k[:, q * seg : (q + 1) * seg], cc_out_h[:])
            else:
                nc.sync.dma_start(s1k[:], s1k_src[:] if use_ag else cc_out[:])
            d1k = io.tile([1, B * N // B], dt.float32, name="d1k", tag="s1k")
            nc.scalar.activation(d1k[:], s1k[:], AF.Copy, bias=1.0, scale=-1.0)
            nc.vector.tensor_scalar(d1k[:], d1k[:], 0.0, 2.0, ALU.max, ALU.min)

            # pred_score: mean of top-10 = (sum(top8) + ranks 9,10) / 10
            top8 = stats.tile([1, 8], dt.float32, name="top8", tag="top8")
            nc.vector.max(top8[:], d1k[:])
            d1kr = io.tile([1, B * N // B], dt.float32, name="d1kr", tag="s1k")
            nc.vector.match_replace(d1kr[:], top8[:], d1k[:], -1e30)
            nxt8 = stats.tile([1, 8], dt.float32, name="nxt8", tag="top8")
            nc.vector.max(nxt8[:], d1kr[:])
            s8 = stats.tile([1, 1], dt.float32, name="s8", tag="acc")
            nc.vector.reduce_sum(s8[:], top8[:], AX.X)
            s2 = stats.tile([1, 1], dt.float32, name="s2", tag="acc")
            nc.vector.reduce_sum(s2[:], nxt8[:, 0:2], AX.X)
            ssum = stats.tile([1, 1], dt.float32, name="ssum", tag="acc")
            nc.vector.tensor_add(ssum[:], s8[:], s2[:])
            sc = stats.tile([1, 1], dt.float32, name="sc", tag="acc")
            nc.scalar.mul(sc[:], ssum[:], 1.0 / NUM_TOP)
            nc.sync.dma_start(score_ext[:], sc[:])

            # anomaly map: out = L @ A @ L^T, via AT then two matmuls with LT
            at_ps = tp_tile([GRID, GRID], dt.float32, "at_ps")
            nc.tensor.transpose(at_ps[:], d32[:], ident_f[0:GRID, 0:GRID])
            at = io.tile([GRID, GRID], dt.float32, name="at", tag="s32")
            nc.scalar.copy(at[:], at_ps[:])
            p_ps = tp_tile([GRID, IMG], dt.float32, "p_ps")
            nc.tensor.matmul(p_ps[:], at[:], lt_sb[:], start=True, stop=True)
            p_sb = io.tile([GRID, IMG], dt.float32, name="p_sb", tag="p_sb")
            nc.scalar.copy(p_sb[:], p_ps[:])
            for it in range(4):
                ow = min(P, IMG - it * P)
                o_ps = tp_tile([P, IMG], dt.float32, "o_ps")
                nc.tensor.matmul(
                    o_ps[0:ow, :],
                    lt_sb[:, it * P : it * P + ow],
                    p_sb[:],
                    start=True,
                    stop=True,
                )
                o_sb = io.tile([P, IMG], dt.float32, name="o_sb", tag="o_sb")
                nc.scalar.copy(o_sb[0:ow, :], o_ps[0:ow, :])
                nc.sync.dma_start(amap_ext[it * P : it * P + ow, :], o_sb[0:ow, :])

    nc.compile()
    return nc


def _in_maps(features: np.ndarray, memory_bank: np.ndarray):
    import os

    f = np.ascontiguousarray(np.asarray(features, dtype=np.float32).reshape(B * N, D))
    mb = np.asarray(memory_bank, dtype=np.float32)
    lt = _build_lt()
    in_maps = [
        {
            "features": f,
            "mb": np.ascontiguousarray(mb[c * MSH : (c + 1) * MSH]),
            "lt": lt,
        }
        for c in range(NCORES)
    ]
    if os.environ.get("KBENCH_AG", "0") == "1":
        for c in range(NCORES):
            sel = np.zeros((NPT, NCORES), dtype=np.float32)
            for i in range(NCORES):
                sel[NCORES * c + i, i] = 1.0
            in_maps[c]["sel"] = sel
    return in_maps


def _run(features: np.ndarray, memory_bank: np.ndarray, trace: bool = False):
    nc = _get_nc()
    from concourse.bass_utils import run_bass_kernel_spmd

    in_maps = _in_maps(features, memory_bank)
    res = run_bass_kernel_spmd(nc, in_maps, core_ids=list(range(NCORES)), trace=trace)
    results = res.results
    pred = np.stack(
        [results[c]["score"].reshape(1) for c in range(NCORES)], axis=0
    ).astype(np.float32)
    amap = np.stack([results[c]["amap"] for c in range(NCORES)], axis=0).reshape(
        B, 1, IMG, IMG
    ).astype(np.float32)
    return (pred, amap), res


def kernel(features: np.ndarray, memory_bank: np.ndarray):
    (pred, amap), _ = _run(features, memory_bank, trace=False)
    return pred, amap
